# revision 1
# baseline (speedup 1.0000x reference)
"""Multi-head attention with KV cache, sharded over 8 NeuronCores by head.

Problem (hardcoded shapes):
  x       [4, 512, 1024]      hidden states (B, T, D)
  k_prev  [4, 16, 3584, 64]   KV cache (B, H, PAST, HD)
  v_prev  [4, 16, 3584, 64]
  Wq/Wk/Wv/Wo [1024, 1024]    projection weights (torch Linear: y = x @ W.T)

Sharding: 16 heads / 8 cores = 2 heads per core (data stays full along batch).
Each core computes q/k/v projections for its 2 heads (column-parallel),
full attention for its heads, and a column-parallel o_proj partial
[2048, 1024]; the host sums the 8 partials (the o_proj all-reduce).

Device algorithm per core (fp32 accumulate, float32r matmul operands --
the TF32-like single-pass PE mode, ~12-13 effective mantissa bits, 4x the
fp32 matmul rate; measured end-to-end rel err ~2e-4):
  - qT/kT_new/vT_new = W_slice @ x^T   via PE, contracting D (xT fed from host)
  - per (batch, head): scores^T[key, q] = k @ q^T (K=HD=64 on partitions),
    streamed in key-chunks of 128 grouped 3 per PSUM tile; the causal mask on
    the 4 newest chunks is accumulated by an extra identity @ mask matmul;
    exp on ScalarE (no max subtraction -- scores are O(1) by construction);
    out^T[hd, q] accumulated in PSUM via lhsT = [v | 1] so row 64 of the
    accumulator is the softmax denominator; divide, then o_proj.
  Phases are software-pipelined: batch-0 KV DMAs and attention overlap the
  projection matmuls, batch b+1 KV setup overlaps batch b, o_proj per batch.
"""

import numpy as np

import concourse.bass as bass
import concourse.mybir as mybir
import concourse.tile as tile
from concourse import bacc
from concourse.bass_utils import run_bass_kernel_spmd
from concourse.masks import make_identity

B, T, D = 4, 512, 1024
H, HD = 16, 64
PAST = 3584
L = PAST + T            # 4096 == MAX_CACHE, nothing is trimmed
SCALE = 1.0 / np.sqrt(HD).astype(np.float32)
NCORES = 8
HPC = H // NCORES       # heads per core = 2
TOK = B * T             # 2048
NCH = L // 128          # 32 key chunks per (b, h)
PCH = PAST // 128       # 28 chunks from the cache
FP32 = mybir.dt.float32
NEG = -1.0e30

_cache = {}

# float32r: 4-byte fp32 variant the PE consumes at full rate (~12-13 mantissa
# bits effective, measured) vs 4 cycles/row for fp32. All matmul operands are
# stored as fp32r; producers (DMA / DVE copy / ACT exp) write the rounded form.
FP32R = mybir.dt.float32r


def _build():
    nc = bacc.Bacc(None, target_bir_lowering=False)

    xT = nc.dram_tensor("xT", [D, TOK], FP32R, kind="ExternalInput")
    wqT = nc.dram_tensor("wqT", [D, 128], FP32R, kind="ExternalInput")
    wkT = nc.dram_tensor("wkT", [D, 128], FP32R, kind="ExternalInput")
    wvT = nc.dram_tensor("wvT", [D, 128], FP32R, kind="ExternalInput")
    woT = nc.dram_tensor("woT", [128, D], FP32R, kind="ExternalInput")
    kTp = nc.dram_tensor("kTp", [B, 128, PAST], FP32R, kind="ExternalInput")
    vp = nc.dram_tensor("vp", [B, 128, HPC, PCH, HD + 1], FP32R, kind="ExternalInput")
    out = nc.dram_tensor("out", [TOK, D], FP32, kind="ExternalOutput")

    Exp = mybir.ActivationFunctionType.Exp
    mult = mybir.AluOpType.mult
    add = mybir.AluOpType.add

    # key-chunk groups: scores psum tiles hold up to 3 chunks (3 PSUM banks)
    groups = [list(range(s, min(s + 3, NCH))) for s in range(0, NCH, 3)]

    with tile.TileContext(nc) as tc:
        with (
            tc.tile_pool(name="const", bufs=1) as const,
            tc.tile_pool(name="persist", bufs=1) as persist,
            tc.tile_pool(name="kv", bufs=2) as kv,
            tc.tile_pool(name="pt", bufs=2) as ptp,
            tc.tile_pool(name="div", bufs=2) as divp,
            tc.tile_pool(name="stage", bufs=1) as stage,
            tc.tile_pool(name="acc_ps", bufs=1, space="PSUM") as accp,
            tc.tile_pool(name="flex_ps", bufs=1, space="PSUM") as flexp,
        ):
            # ---- constants ----
            identity = const.tile([128, 128], FP32)
            make_identity(nc, identity)
            identity_r = const.tile([128, 128], FP32R)
            nc.vector.tensor_copy(identity_r, identity)
            masks = []
            for r in range(4):
                m = const.tile([128, T], FP32, tag=f"mask{r}")
                nc.gpsimd.memset(m, 0.0)
                # keep 0 where query i >= key-token (128r + kk), else NEG
                nc.gpsimd.affine_select(
                    out=m, in_=m, compare_op=mybir.AluOpType.is_ge,
                    fill=NEG, base=-128 * r, channel_multiplier=-1,
                    pattern=[[1, T]],
                )
                mr = const.tile([128, T], FP32R, tag=f"maskr{r}", name=f"maskr{r}")
                nc.vector.tensor_copy(mr, m)
                masks.append(mr)

            ones_c = const.tile([128, 1], FP32)
            nc.gpsimd.memset(ones_c, 1.0)
            warm = const.tile([1, 1], FP32)
            nc.scalar.activation(warm, ones_c[:1, :], Exp)
            ones_r = const.tile([1, HD], FP32R)
            nc.vector.tensor_copy(ones_r, ones_c[:1, :].to_broadcast([1, HD]))

            # ---- persistent SBUF ----
            woT_s = persist.tile([128, D], FP32R)
            qT = persist.tile([128, TOK], FP32R, tag="qT")
            kTn = persist.tile([128, TOK], FP32, tag="kTn")
            vTn = persist.tile([128, TOK], FP32, tag="vTn")
            oT = persist.tile([128, TOK], FP32R, tag="oT")

            def setup_batch(b, kT=None, va=None):
                bsl = bass.ts(b, T)
                if kT is None:
                    kT = kv.tile([128, L], FP32R, tag="kT", name=f"kT{b}")
                    nc.sync.dma_start(kT[:, :PAST], kTp[b, :, :])
                nc.vector.tensor_copy(kT[:, PAST:], kTn[:, bsl])
                if va is None:
                    va = kv.tile(
                        [128, HPC, NCH, HD + 1], FP32R, tag="va", name=f"va{b}"
                    )
                    nc.sync.dma_start(va[:, :, :PCH, :], vp[b, :, :, :, :])
                nc.vector.tensor_copy(
                    va[:, :, PCH:, HD],
                    ones_c[:, :, None].to_broadcast([128, HPC, NCH - PCH]),
                )
                for h in range(HPC):
                    hsl = slice(h * HD, (h + 1) * HD)
                    for tt in range(T // 128):
                        tp = flexp.tile([128, 512], FP32, tag="flex")
                        nc.tensor.transpose(
                            tp[:, :HD],
                            vTn[hsl, b * T + tt * 128 : b * T + (tt + 1) * 128],
                            identity[hsl, hsl],
                        )
                        nc.vector.tensor_copy(va[:, h, PCH + tt, :HD], tp[:, :HD])
                return kT, va

            # ---- phase A: projections (q/k/v for this core's 2 heads) ----
            nxt = None
            with (
                tc.tile_pool(name="xw", bufs=1) as xw,
                tc.tile_pool(name="xs", bufs=1) as xs,
            ):
                xT_r = xT.rearrange("(ko p) t -> p ko t", p=128)
                w_s = {}
                for name, w in (("q", wqT), ("k", wkT), ("v", wvT)):
                    w_s[name] = xw.tile(
                        [128, D // 128, 128], FP32R, tag=f"w{name}", name=f"w{name}"
                    )
                    if name == "q":
                        nc.sync.dma_start(
                            w_s[name], w.rearrange("(ko p) m -> p ko m", p=128)
                        )
                        xT_s0 = xs.tile([128, D // 128, 512], FP32R, tag="xT")
                        half = D // 256
                        nc.sync.dma_start(
                            xT_s0[:, :half, :], xT_r[:, :half, :512]
                        )
                        nc.sync.dma_start(
                            xT_s0[:, half:, :], xT_r[:, half:, :512]
                        )
                kT0 = kv.tile([128, L], FP32R, tag="kT", name="kT0")
                nc.sync.dma_start(kT0[:, : 12 * 128], kTp[0, :, : 12 * 128])
                va0 = kv.tile(
                    [128, HPC, NCH, HD + 1], FP32R, tag="va", name="va0"
                )
                nc.sync.dma_start(va0[:, :, :12, :], vp[0, :, :, :12, :])
                for name, w in (("k", wkT), ("v", wvT)):
                    nc.sync.dma_start(
                        w_s[name], w.rearrange("(ko p) m -> p ko m", p=128)
                    )
                nc.sync.dma_start(kT0[:, 12 * 128 : PAST], kTp[0, :, 12 * 128 :])
                nc.sync.dma_start(va0[:, :, 12:PCH, :], vp[0, :, :, 12:, :])
                def proj_tc(tcn, xT_s=None):
                    if xT_s is None:
                        xT_s = xs.tile(
                            [128, D // 128, 512], FP32R, tag="xT", name="xT_s"
                        )
                        half = D // 256
                        nc.sync.dma_start(
                            xT_s[:, :half, :], xT_r[:, :half, bass.ts(tcn, 512)]
                        )
                        nc.sync.dma_start(
                            xT_s[:, half:, :], xT_r[:, half:, bass.ts(tcn, 512)]
                        )
                    for name, dst in (("q", qT), ("k", kTn), ("v", vTn)):
                        ps = flexp.tile([128, 512], FP32, tag="flex")
                        for ko in range(D // 128):
                            nc.tensor.matmul(
                                ps,
                                lhsT=w_s[name][:, ko, :],
                                rhs=xT_s[:, ko, :],
                                start=(ko == 0),
                                stop=(ko == D // 128 - 1),
                            )
                        nc.vector.tensor_copy(dst[:, bass.ts(tcn, 512)], ps)

                proj_tc(0, xT_s=xT_s0)
                nxt = setup_batch(0, kT=kT0, va=va0)
                proj_tc(1)

                nc.sync.dma_start(woT_s, woT[:, :])

                # ---- phase B: attention per (batch, head) ----
                scp_cm = tc.tile_pool(name="sc_ps", bufs=2, space="PSUM")
                scp = scp_cm.__enter__()
                for b in range(B):
                    bsl = bass.ts(b, T)
                    kT, va = nxt
                    if b + 2 < B:
                        proj_tc(b + 2)
                    if b + 1 < B:
                        nxt = setup_batch(b + 1)

                    for h in range(HPC):
                        hsl = slice(h * HD, (h + 1) * HD)
                        acc = accp.tile([HD + 1, 512], FP32, tag="acc")
                        qTh = qT[hsl, bsl]
                        for g in groups:
                            ng = len(g)
                            ps = scp.tile([128, 3 * 512], FP32, tag="sc")
                            for j, cc in enumerate(g):
                                masked = cc >= PCH
                                # queries < off see nothing from chunk cc
                                off = max(0, (cc - PCH) * 128)
                                nc.tensor.matmul(
                                    ps[:, j * 512 + off : (j + 1) * 512],
                                    lhsT=kT[hsl, bass.ts(cc, 128)],
                                    rhs=qTh[:, off:],
                                    start=True,
                                    stop=not masked,
                                )
                                if masked:
                                    nc.tensor.matmul(
                                        ps[:, j * 512 + off : (j + 1) * 512],
                                        lhsT=identity_r,
                                        rhs=masks[cc - PCH][:, off:],
                                        start=False,
                                        stop=True,
                                        skip_group_check=True,
                                    )
                            pT = ptp.tile([128, 3 * 512], FP32R, tag="pT")
                            nc.scalar.activation(
                                pT[:, : ng * 512], ps[:, : ng * 512], Exp
                            )
                            for j, cc in enumerate(g):
                                off = max(0, (cc - PCH) * 128)
                                nc.tensor.matmul(
                                    acc[:, off:],
                                    lhsT=va[:, h, cc, :],
                                    rhs=pT[:, j * 512 + off : (j + 1) * 512],
                                    start=(cc == 0),
                                    stop=(cc == NCH - 1),
                                    skip_group_check=True,
                                )
                        # evict accumulator to SBUF at once (frees the PSUM
                        # bank for the next head); denominator in row 64
                        asb = divp.tile([HD + 1, 512], FP32, tag="asb")
                        nc.vector.tensor_copy(asb, acc)
                        if b == B - 1 and h == HPC - 1:
                            r0r = divp.tile([1, 512], FP32R, tag="r0r")
                            with nc.allow_low_precision(
                                reason="fp32r reciprocal feeds broadcast matmul"
                            ):
                                nc.vector.reciprocal(r0r, asb[HD : HD + 1, :])
                            bcp = flexp.tile(
                                [HD, 512], FP32, tag="flex", name="bcp"
                            )
                            nc.tensor.matmul(
                                bcp, lhsT=ones_r, rhs=r0r, start=True, stop=True
                            )
                            nc.vector.tensor_tensor(
                                oT[hsl, bsl], asb[:HD, :], bcp, mult
                            )
                        else:
                            r0 = divp.tile([1, 512], FP32, tag="r0")
                            nc.vector.reciprocal(r0, asb[HD : HD + 1, :])
                            bc = divp.tile([HD, 512], FP32, tag="bc")
                            nc.gpsimd.partition_broadcast(bc, r0)
                            nc.vector.tensor_tensor(
                                oT[hsl, bsl], asb[:HD, :], bc, mult
                            )

                    # ---- column-parallel o_proj for this batch ----
                    out_r = out[bsl, :].rearrange("(tt p) d -> p tt d", p=128)
                    if b == B - 1:
                        ostl = stage.tile(
                            [128, T // 128, D], FP32, tag="ost", name="ostl"
                        )
                        for tt in range(T // 128):
                            tsl = slice(b * T + tt * 128, b * T + (tt + 1) * 128)
                            for nh in range(2):
                                ps = scp.tile([128, 3 * 512], FP32, tag="sc")
                                ps = ps[:, :512]
                                nc.tensor.matmul(
                                    ps,
                                    lhsT=oT[:, tsl],
                                    rhs=woT_s[:, bass.ts(nh, 512)],
                                    start=True,
                                    stop=True,
                                )
                                if nh == 1:
                                    nc.scalar.copy(
                                        ostl[:, tt, bass.ts(nh, 512)], ps
                                    )
                                else:
                                    nc.vector.tensor_copy(
                                        ostl[:, tt, bass.ts(nh, 512)], ps
                                    )
                            nc.sync.dma_start(out_r[:, tt, :], ostl[:, tt, :])
                    else:
                        ost = stage.tile([128, T // 128, D], FP32, tag="ost")
                        for tt in range(T // 128):
                            tsl = slice(b * T + tt * 128, b * T + (tt + 1) * 128)
                            for nh in range(2):
                                ps = flexp.tile([128, 512], FP32, tag="flex")
                                nc.tensor.matmul(
                                    ps,
                                    lhsT=oT[:, tsl],
                                    rhs=woT_s[:, bass.ts(nh, 512)],
                                    start=True,
                                    stop=True,
                                )
                                nc.vector.tensor_copy(
                                    ost[:, tt, bass.ts(nh, 512)], ps
                                )
                            nc.sync.dma_start(out_r[:, tt, :], ost[:, tt, :])
                scp_cm.__exit__(None, None, None)

    nc.compile()
    return nc


def _pack_v(v):
    """[B, HPC, PAST, HD] -> [B, 128, HPC, PCH, HD+1] with ones in col HD."""
    out = np.empty((B, 128, HPC, PCH, HD + 1), np.float32)
    # v[b, h, c*128 + p, hd] -> out[b, p, h, c, hd]
    out[..., :HD] = v.reshape(B, HPC, PCH, 128, HD).transpose(0, 3, 1, 2, 4)
    out[..., HD] = 1.0
    return np.ascontiguousarray(out)


def _prep(x, k_prev, v_prev, Wq, Wk, Wv, Wo):
    """Host-side shard + layout marshalling (fp32, C-contiguous)."""
    f = np.float32
    x2 = np.ascontiguousarray(np.asarray(x, f).reshape(TOK, D))
    xT = np.ascontiguousarray(x2.T)
    k_prev = np.asarray(k_prev, f)
    v_prev = np.asarray(v_prev, f)
    Wq, Wk, Wv, Wo = (np.asarray(w, f) for w in (Wq, Wk, Wv, Wo))
    in_maps = []
    for c in range(NCORES):
        rows = slice(128 * c, 128 * (c + 1))
        hsl = slice(HPC * c, HPC * (c + 1))
        in_maps.append(
            {
                "xT": xT,
                "wqT": np.ascontiguousarray((Wq[rows, :] * SCALE).T),
                "wkT": np.ascontiguousarray(Wk[rows, :].T),
                "wvT": np.ascontiguousarray(Wv[rows, :].T),
                "woT": np.ascontiguousarray(Wo[:, rows].T),
                "kTp": np.ascontiguousarray(
                    k_prev[:, hsl, :, :].transpose(0, 1, 3, 2)
                ).reshape(B, 128, PAST),
                "vp": _pack_v(v_prev[:, hsl, :, :]),
            }
        )
    return in_maps


def kernel(x, k_prev, v_prev, Wq, Wk, Wv, Wo):
    if "nc" not in _cache:
        _cache["nc"] = _build()
    nc = _cache["nc"]
    in_maps = _prep(x, k_prev, v_prev, Wq, Wk, Wv, Wo)
    res = run_bass_kernel_spmd(nc, in_maps, core_ids=list(range(NCORES)))
    acc = np.zeros((TOK, D), np.float64)
    for r in res.results:
        acc += r["out"]
    return acc.astype(np.float32).reshape(B, T, D)



# revision 25
# speedup vs baseline: 1.2978x; 1.2978x over previous
"""Multi-head attention with KV cache, sharded over 8 NeuronCores by head.

Problem (hardcoded shapes):
  x       [4, 512, 1024]      hidden states (B, T, D)
  k_prev  [4, 16, 3584, 64]   KV cache (B, H, PAST, HD)
  v_prev  [4, 16, 3584, 64]
  Wq/Wk/Wv/Wo [1024, 1024]    projection weights (torch Linear: y = x @ W.T)

Sharding: 16 heads / 8 cores = 2 heads per core (data stays full along batch).
Each core computes q/k/v projections for its 2 heads (column-parallel),
full attention for its heads, and a column-parallel o_proj partial
[2048, 1024] in fp16; the host sums the 8 partials (the o_proj all-reduce).

Device algorithm per core (fp16 matmul operands, fp32 PSUM accumulate,
measured end-to-end rel err ~5e-4):
  - q/k projections: W_slice @ x^T on PE, contracting D; evicted fp16 into
    qT [128, TOK] and per-batch k caches kT_b [128, L] (cache DMA'd fp16).
  - v projection computed PRE-TRANSPOSED (out[token, hd] per 128-token tile)
    directly into the va value cache [128keys, 2h, 32chunk, 65] whose 65th
    column is 1.0 (softmax denominator rides the AV matmul).
  - scores^T[key, q] = k @ q^T per 128-key chunk (K=HD=64), two chunks per
    2-bank PSUM pair tile; causal mask on the 4 newest chunks is accumulated
    via an e5m2 DoubleRow matmul (stacked-identity weights, mask = -256,
    exact in e5m2, cost half of an fp16 matmul).
  - softmax: exp on ScalarE with scale=1/sqrt(HD) folded in (scores are O(1);
    no max subtraction), fp16 out. Optionally a fraction of chunk-pairs use
    a 1-op Schraudolph exp on DVE (int16-bitcast fp16) to offload ScalarE.
  - AV TRANSPOSED: per (chunk, 128-query tile): acc[q, 0:65] += pT_tile^T @
    [v|1]  -- all four query tiles accumulate in ONE psum bank, N=65 per
    matmul so PE cost is half of the straight orientation.  Divide is then a
    per-partition reciprocal+multiply (denominator in column 64), transposed
    back to oT [hd, tok] by a PE transpose (fp16).
  - o_proj column-parallel per batch; fp16 partial [2048, 1024] written out.
  Phases are software-pipelined: per-batch KV cache DMAs and projections for
  batch b+2 overlap attention on batch b; o_proj per batch.
"""

import numpy as np
import ml_dtypes

import concourse.bass as bass
import concourse.mybir as mybir
import concourse.tile as tile
from concourse import bacc
from concourse.bass_utils import run_bass_kernel_spmd
from concourse.masks import make_identity

B, T, D = 4, 512, 1024
H, HD = 16, 64
PAST = 3584
L = PAST + T            # 4096 == MAX_CACHE, nothing is trimmed
SCALE = float(1.0 / np.sqrt(HD))
NCORES = 8
HPC = H // NCORES       # heads per core = 2
TOK = B * T             # 2048
NCH = L // 128          # 32 key chunks per (b, h)

PCH = PAST // 128       # 28 chunks from the cache
NPAIR = NCH // 2        # 16 chunk pairs (one exp instruction each)
FP32 = mybir.dt.float32
FP16 = mybir.dt.float16
E5 = mybir.dt.float8e5
I16 = mybir.dt.int16
NEG = -256.0            # mask added in psum units; exp(-256/8) == 0 in fp16
F16NP = np.float16

# Every third (chunk, head) slot computes softmax exp on DVE via a
# bias-corrected Schraudolph approximation (int16 rint -> bitcast fp16,
# ~1.8% rms multiplicative ripple); the rest use true exp on ScalarE.
# At key-fraction 1/3 this adds ~8e-3 end-to-end rel err (gate is 2e-2).
SCH_A = float((1024.0 / np.log(2.0)) * SCALE)
SCH_B = 15360.0 - 59.6

_cache = {}


def _build():
    nc = bacc.Bacc(None, target_bir_lowering=False)

    xT = nc.dram_tensor("xT", [D, TOK], FP16, kind="ExternalInput")
    wq = nc.dram_tensor("wq", [128, D // 128, 128], FP16, kind="ExternalInput")
    wk = nc.dram_tensor("wk", [128, D // 128, 128], FP16, kind="ExternalInput")
    wv = nc.dram_tensor("wv", [128, D // 128, 128], FP16, kind="ExternalInput")
    woT = nc.dram_tensor("woT", [128, D], FP16, kind="ExternalInput")
    kTp = nc.dram_tensor("kTp", [B, 128, PAST], FP16, kind="ExternalInput")
    vp = nc.dram_tensor("vp", [B, 128, HPC, PCH, HD + 1], FP16, kind="ExternalInput")
    out = nc.dram_tensor("out", [TOK, D], FP16, kind="ExternalOutput")

    Exp = mybir.ActivationFunctionType.Exp
    mult = mybir.AluOpType.mult
    add = mybir.AluOpType.add
    DR = mybir.MatmulPerfMode.DoubleRow

    with tile.TileContext(nc) as tc:
        with (
            tc.tile_pool(name="const", bufs=1) as const,
            tc.tile_pool(name="persist", bufs=1) as persist,
            tc.tile_pool(name="xs", bufs=2) as xs,
            tc.tile_pool(name="pta", bufs=9) as ptap,
            tc.tile_pool(name="ptd", bufs=6) as ptdp,
            tc.tile_pool(name="ott", bufs=4) as ottp,
            tc.tile_pool(name="ost", bufs=3) as ostp,
            tc.tile_pool(name="sc_ps", bufs=4, space="PSUM") as scp,
            tc.tile_pool(name="acc_ps", bufs=2, space="PSUM") as accp,
            tc.tile_pool(name="flex_ps", bufs=2, space="PSUM") as flexp,
        ):
            # ---- constants ----
            identity = const.tile([128, 128], FP32)
            make_identity(nc, identity)
            id16 = const.tile([128, 128], FP16)
            nc.vector.tensor_copy(id16, identity)
            # fp16 causal masks, applied via an identity matmul accumulated
            # into the scores group (dtype must match the scores matmul: a
            # mid-group dtype/perf-mode switch faults the PE).
            # maskk[p, r, t] = NEG if t < 128r + p else 0
            maskk = const.tile([128, 4, T], FP16)
            nc.gpsimd.memset(maskk, 0.0)
            for r in range(4):
                nc.gpsimd.affine_select(
                    out=maskk[:, r, :], in_=maskk[:, r, :],
                    compare_op=mybir.AluOpType.is_ge,
                    fill=NEG, base=-(128 * r),
                    channel_multiplier=-1, pattern=[[1, T]],
                )
            ones_c = const.tile([128, 1], FP16)
            nc.gpsimd.memset(ones_c, 1.0)
            warm = const.tile([1, 1], FP32)
            nc.scalar.activation(warm, identity[:1, :1], Exp)

            # ---- persistent SBUF ----
            woT_s = persist.tile([128, D], FP16)
            qT = persist.tile([128, TOK], FP16, tag="qT")
            oT = persist.tile([128, TOK], FP16, tag="oT")
            w_s = {}
            for name, w in (("q", wq), ("k", wk), ("v", wv)):
                w_s[name] = persist.tile(
                    [128, D // 128, 128], FP16, tag=f"w{name}", name=f"w{name}"
                )
            kT_b = [
                persist.tile([128, L], FP16, tag=f"kT{b}", name=f"kT{b}")
                for b in range(B)
            ]
            va_b = [
                persist.tile([128, HPC, NCH, HD + 1], FP16, tag=f"va{b}",
                             name=f"va{b}")
                for b in range(B)
            ]

            xT_r = xT.rearrange("(ko p) t -> p ko t", p=128)

            def dma_cache(b):
                nc.sync.dma_start(kT_b[b][:, :PAST], kTp[b, :, :])
                nc.sync.dma_start(va_b[b][:, :, :PCH, :], vp[b, :, :, :, :])
                # ones column for the 4 new-v chunks
                nc.vector.tensor_copy(
                    va_b[b][:, :, PCH:, HD],
                    ones_c[:, :, None].to_broadcast([128, HPC, NCH - PCH]),
                )

            def dma_x(b, xT_s=None):
                if xT_s is None:
                    xT_s = xs.tile([128, D // 128, 512], FP16, tag="xT")
                half = D // 256
                nc.sync.dma_start(xT_s[:, :half, :], xT_r[:, :half, bass.ts(b, 512)])
                nc.sync.dma_start(xT_s[:, half:, :], xT_r[:, half:, bass.ts(b, 512)])
                return xT_s

            def proj_qk(b, xT_s, name):
                dst = qT[:, bass.ts(b, T)] if name == "q" else kT_b[b][:, PAST:]
                ps = flexp.tile([128, 512], FP32, tag="flex")
                for ko in range(D // 128):
                    nc.tensor.matmul(
                        ps, lhsT=w_s[name][:, ko, :], rhs=xT_s[:, ko, :],
                        start=(ko == 0), stop=(ko == D // 128 - 1),
                    )
                nc.vector.tensor_copy(dst, ps)

            def proj_v(b, xT_s, tt):
                ps = flexp.tile([128, 512], FP32, tag="flex")
                for ko in range(D // 128):
                    nc.tensor.matmul(
                        ps[:, :128],
                        lhsT=xT_s[:, ko, bass.ts(tt, 128)],
                        rhs=w_s["v"][:, ko, :],
                        start=(ko == 0), stop=(ko == D // 128 - 1),
                    )
                for h in range(HPC):
                    nc.vector.tensor_copy(
                        va_b[b][:, h, PCH + tt, :HD],
                        ps[:, h * HD:(h + 1) * HD],
                    )

            def proj_pieces(b, xT_s):
                return [
                    lambda: proj_qk(b, xT_s, "q"),
                    lambda: proj_qk(b, xT_s, "k"),
                ] + [
                    (lambda tt: lambda: proj_v(b, xT_s, tt))(tt)
                    for tt in range(T // 128)
                ]

            def proj(b, xT_s):
                for piece in proj_pieces(b, xT_s):
                    piece()

            # ---- phase A: caches + projections for b0/b1 ----
            nc.sync.dma_start(w_s["q"], wq[:, :, :])
            xT_s0 = dma_x(0)
            nc.sync.dma_start(w_s["k"], wk[:, :, :])
            nc.sync.dma_start(w_s["v"], wv[:, :, :])
            dma_cache(0)
            xT_s1 = dma_x(1)
            dma_cache(1)
            proj(0, xT_s0)
            proj(1, xT_s1)
            nc.sync.dma_start(woT_s, woT[:, :])

            # ---- phase B: attention, both heads' chunk streams interleaved ----
            # One score chunk per 1-bank psum tile (ring of 4); each chunk's
            # softmax exp is ONE instruction on ONE engine (PSUM dep tracking
            # is bank-granular, so any split of a tile across engines would
            # serialize them). Chunks go 2:1 to ScalarE (true exp) : DVE
            # (Schraudolph); AV matmuls are deferred several slots so they
            # never stall PE's score stream.
            def attn(b, filler):
                bsl = bass.ts(b, T)
                kT = kT_b[b]
                va = va_b[b]
                accs = [
                    accp.tile([128, 512], FP32, tag="acc", name=f"acc{b}_{h}")
                    for h in range(HPC)
                ]

                def av(c, h, pT16):
                    qt0 = max(0, c - PCH)  # first query tile this chunk sees
                    for qt in range(qt0, 4):
                        # One accumulation group for the whole bank: HW (like
                        # the sim) zeroes the full 2KB zero-region on the first
                        # start=True and lazily zero-fills each byte's first
                        # write, so all four qt sub-ranges share the group.
                        nc.tensor.matmul(
                            accs[h][:, qt * 65:qt * 65 + 65],
                            lhsT=pT16[:, bass.ts(qt, 128)],
                            rhs=va[:, h, c, :],
                            start=(c == 0 and qt == 0),
                            stop=(c == NCH - 1 and qt == 3),
                            skip_group_check=True,
                        )

                pend = []
                slot = 0
                for c in range(NCH):
                    for h in range(HPC):
                        hsl = slice(h * HD, (h + 1) * HD)
                        off = max(0, (c - PCH) * 128)
                        masked = c >= PCH
                        S = scp.tile([128, 512], FP32, tag="sc")
                        nc.tensor.matmul(
                            S[:, off:],
                            lhsT=kT[hsl, bass.ts(c, 128)],
                            rhs=qT[hsl, bsl][:, off:],
                            start=True, stop=not masked,
                        )
                        if masked:
                            nc.tensor.matmul(
                                S[:, off:],
                                lhsT=id16,
                                rhs=maskk[:, c - PCH, off:],
                                start=False, stop=True,
                                skip_group_check=True,
                            )
                        if len(pend) >= 7:
                            av(*pend.pop(0))
                        if slot % 3 == 2:  # DVE schraudolph exp
                            pTd = ptdp.tile([128, 512], I16, tag="pTd")
                            nc.vector.tensor_scalar(
                                pTd[:, off:], S[:, off:], SCH_A, SCH_B,
                                op0=mult, op1=add,
                            )
                            pT16 = pTd.bitcast(FP16)
                        else:  # ScalarE true exp
                            pT16 = ptap.tile([128, 512], FP16, tag="pTa")
                            nc.scalar.activation(
                                pT16[:, off:], S[:, off:], Exp, scale=SCALE
                            )
                        pend.append((c, h, pT16))
                        if filler and slot % 6 == 3:
                            filler.pop(0)()
                        slot += 1
                for pp in pend:
                    av(*pp)
                while filler:
                    filler.pop(0)()
                # divide (denominator in col 64 of each qt block), transpose to oT
                for h in range(HPC):
                    hsl = slice(h * HD, (h + 1) * HD)
                    for qt in range(4):
                        a = accs[h][:, qt * 65:qt * 65 + 65]
                        r = ottp.tile([128, 1], FP32, tag="r")
                        nc.vector.reciprocal(r, a[:, 64:65])
                        ot = ottp.tile([128, 64], FP16, tag="ott")
                        nc.vector.tensor_scalar(ot, a[:, :64], r, None, op0=mult)
                        tp = flexp.tile([64, 128], FP16, tag="flex")
                        nc.tensor.transpose(tp, ot, id16)
                        nc.vector.tensor_copy(
                            oT[hsl, b * T + qt * 128:b * T + (qt + 1) * 128], tp
                        )

            def o_proj_piece(b, tt):
                out_r = out[bass.ts(b, T), :].rearrange("(tt p) d -> p tt d", p=128)
                tsl = slice(b * T + tt * 128, b * T + (tt + 1) * 128)
                ost = ostp.tile([128, D], FP16, tag="ost")
                for nh in range(2):
                    ps = flexp.tile([128, 512], FP32, tag="flex")
                    nc.tensor.matmul(
                        ps, lhsT=oT[:, tsl], rhs=woT_s[:, bass.ts(nh, 512)],
                        start=True, stop=True,
                    )
                    nc.vector.tensor_copy(ost[:, bass.ts(nh, 512)], ps)
                nc.sync.dma_start(out_r[:, tt, :], ost)

            def o_proj_pieces(b):
                return [
                    (lambda tt: lambda: o_proj_piece(b, tt))(tt)
                    for tt in range(T // 128)
                ]

            for b in range(B):
                filler = []
                if b + 2 < B:
                    xT_s = dma_x(b + 2)
                    dma_cache(b + 2)
                    filler += proj_pieces(b + 2, xT_s)
                if b > 0:
                    filler += o_proj_pieces(b - 1)
                attn(b, filler)
            for piece in o_proj_pieces(B - 1):
                piece()

    nc.compile()
    return nc


def _prep(x, k_prev, v_prev, Wq, Wk, Wv, Wo):
    """Host-side shard + fp16 layout marshalling."""
    f = np.float32
    x2 = np.ascontiguousarray(np.asarray(x, f).reshape(TOK, D))
    xT = np.ascontiguousarray(x2.T).astype(F16NP)
    k_prev = np.asarray(k_prev, f)
    v_prev = np.asarray(v_prev, f)
    Wq, Wk, Wv, Wo = (np.asarray(w, f) for w in (Wq, Wk, Wv, Wo))

    def wpack(Wrows):  # [128, D] -> [128dp, ko, 128m]: w[dp,ko,m] = W[m, 128ko+dp]
        return np.ascontiguousarray(
            Wrows.T.reshape(D // 128, 128, 128).transpose(1, 0, 2)
        ).astype(F16NP)

    in_maps = []
    for c in range(NCORES):
        rows = slice(128 * c, 128 * (c + 1))
        hsl = slice(HPC * c, HPC * (c + 1))
        kT = np.ascontiguousarray(
            k_prev[:, hsl, :, :].transpose(0, 1, 3, 2)
        ).reshape(B, 128, PAST).astype(F16NP)
        vpk = np.empty((B, 128, HPC, PCH, HD + 1), F16NP)
        vpk[..., :HD] = v_prev[:, hsl, :, :].reshape(
            B, HPC, PCH, 128, HD
        ).transpose(0, 3, 1, 2, 4).astype(F16NP)
        vpk[..., HD] = 1.0
        in_maps.append(
            {
                "xT": xT,
                "wq": wpack(Wq[rows, :]),
                "wk": wpack(Wk[rows, :]),
                "wv": wpack(Wv[rows, :]),
                "woT": np.ascontiguousarray(Wo[:, rows].T).astype(F16NP),
                "kTp": kT,
                "vp": np.ascontiguousarray(vpk),
            }
        )
    return in_maps


def kernel(x, k_prev, v_prev, Wq, Wk, Wv, Wo):
    if "nc" not in _cache:
        _cache["nc"] = _build()
    nc = _cache["nc"]
    in_maps = _prep(x, k_prev, v_prev, Wq, Wk, Wv, Wo)
    res = run_bass_kernel_spmd(nc, in_maps, core_ids=list(range(NCORES)))
    acc = np.zeros((TOK, D), np.float64)
    for r in res.results:
        acc += r["out"]
    return acc.astype(np.float32).reshape(B, T, D)


# revision 28
# speedup vs baseline: 1.3024x; 1.0036x over previous
"""Multi-head attention with KV cache, sharded over 8 NeuronCores by head.

Problem (hardcoded shapes):
  x       [4, 512, 1024]      hidden states (B, T, D)
  k_prev  [4, 16, 3584, 64]   KV cache (B, H, PAST, HD)
  v_prev  [4, 16, 3584, 64]
  Wq/Wk/Wv/Wo [1024, 1024]    projection weights (torch Linear: y = x @ W.T)

Sharding: 16 heads / 8 cores = 2 heads per core (data stays full along batch).
Each core computes q/k/v projections for its 2 heads (column-parallel),
full attention for its heads, and a column-parallel o_proj partial
[2048, 1024] in fp16; the host sums the 8 partials (the o_proj all-reduce).

Device algorithm per core (fp16 matmul operands, fp32 PSUM accumulate,
measured end-to-end rel err ~8e-3 incl. the partial Schraudolph softmax):
  - q/k projections: W_slice @ x^T on PE, contracting D; evicted fp16 into
    qT [128, TOK] and per-batch k caches kT_b [128, L] (cache DMA'd fp16).
  - v projection computed PRE-TRANSPOSED (out[token, hd] per 128-token tile)
    directly into the va value cache [128keys, 2h, 32chunk, 65] whose 65th
    column is 1.0 (softmax denominator rides the AV matmul).
  - scores^T[key, q] = k @ q^T per 128-key chunk (K=HD=64), one chunk per
    1-bank PSUM tile on a ring of 4; both heads' chunk streams interleave so
    two softmax chains are always in flight.  Causal mask on the 4 newest
    chunks accumulates an fp16 identity @ mask matmul into the same group
    (dtype must match the scores matmul: a mid-group dtype or perf-mode
    switch faults the PE).
  - softmax: one exp instruction per chunk on ONE engine (PSUM dependency
    tracking is bank-granular; splitting a tile across engines serializes
    them).  ~11/16 of chunks use true exp on ScalarE (scale=1/sqrt(HD)
    folded in; scores are O(1), no max subtraction), 5/16 use a one-op
    bias-corrected Schraudolph exp on DVE (rint to int16, bitcast fp16,
    ~1.8% rms ripple that largely cancels in the softmax ratio).
  - AV TRANSPOSED: per (chunk, 128-query tile): acc[q, 0:65] += pT_tile^T @
    [v|1] -- all four query tiles accumulate in ONE psum bank as a single
    accumulation group (HW zeroes the 2KB zero-region on the first
    start=True and lazily zero-fills each byte's first write), N=65 per
    matmul so PE cost is half of the straight orientation.  AV matmuls are
    deferred 7 slots so PE's score stream never stalls on exp.  Divide is a
    per-partition reciprocal+multiply (denominator in column 64), then a
    PE transpose (fp16) back to oT [hd, tok].
  - o_proj column-parallel per batch; fp16 partial [2048, 1024] written out.
  Projections for batch b+2 and o_proj for batch b-1 are emitted as filler
  pieces inside batch b's attention loop to fill PE gaps.
"""

import numpy as np

import concourse.bass as bass
import concourse.mybir as mybir
import concourse.tile as tile
from concourse import bacc
from concourse.bass_utils import run_bass_kernel_spmd
from concourse.masks import make_identity

B, T, D = 4, 512, 1024
H, HD = 16, 64
PAST = 3584
L = PAST + T            # 4096 == MAX_CACHE, nothing is trimmed
SCALE = float(1.0 / np.sqrt(HD))
NCORES = 8
HPC = H // NCORES       # heads per core = 2
TOK = B * T             # 2048
NCH = L // 128          # 32 key chunks per (b, h)

PCH = PAST // 128       # 28 chunks from the cache
NPAIR = NCH // 2        # 16 chunk pairs (one exp instruction each)
FP32 = mybir.dt.float32
FP16 = mybir.dt.float16
I16 = mybir.dt.int16
NEG = -256.0            # mask added in psum units; exp(-256/8) == 0 in fp16
F16NP = np.float16

# Every third (chunk, head) slot computes softmax exp on DVE via a
# bias-corrected Schraudolph approximation (int16 rint -> bitcast fp16,
# ~1.8% rms multiplicative ripple); the rest use true exp on ScalarE.
# At key-fraction 1/3 this adds ~8e-3 end-to-end rel err (gate is 2e-2).
SCH_A = float((1024.0 / np.log(2.0)) * SCALE)
SCH_B = 15360.0 - 59.6

_cache = {}


def _build():
    nc = bacc.Bacc(None, target_bir_lowering=False)

    xT = nc.dram_tensor("xT", [D, TOK], FP16, kind="ExternalInput")
    wq = nc.dram_tensor("wq", [128, D // 128, 128], FP16, kind="ExternalInput")
    wk = nc.dram_tensor("wk", [128, D // 128, 128], FP16, kind="ExternalInput")
    wv = nc.dram_tensor("wv", [128, D // 128, 128], FP16, kind="ExternalInput")
    woT = nc.dram_tensor("woT", [128, D], FP16, kind="ExternalInput")
    kTp = nc.dram_tensor("kTp", [B, 128, PAST], FP16, kind="ExternalInput")
    vp = nc.dram_tensor("vp", [B, 128, HPC, PCH, HD + 1], FP16, kind="ExternalInput")
    out = nc.dram_tensor("out", [TOK, D], FP16, kind="ExternalOutput")

    Exp = mybir.ActivationFunctionType.Exp
    mult = mybir.AluOpType.mult
    add = mybir.AluOpType.add

    with tile.TileContext(nc) as tc:
        with (
            tc.tile_pool(name="const", bufs=1) as const,
            tc.tile_pool(name="persist", bufs=1) as persist,
            tc.tile_pool(name="xs", bufs=2) as xs,
            tc.tile_pool(name="pta", bufs=9) as ptap,
            tc.tile_pool(name="ptd", bufs=6) as ptdp,
            tc.tile_pool(name="ott", bufs=4) as ottp,
            tc.tile_pool(name="ost", bufs=3) as ostp,
            tc.tile_pool(name="sc_ps", bufs=4, space="PSUM") as scp,
            tc.tile_pool(name="acc_ps", bufs=2, space="PSUM") as accp,
            tc.tile_pool(name="flex_ps", bufs=2, space="PSUM") as flexp,
        ):
            # ---- constants ----
            identity = const.tile([128, 128], FP32)
            make_identity(nc, identity)
            id16 = const.tile([128, 128], FP16)
            nc.vector.tensor_copy(id16, identity)
            # fp16 causal masks, applied via an identity matmul accumulated
            # into the scores group (dtype must match the scores matmul: a
            # mid-group dtype/perf-mode switch faults the PE).
            # maskk[p, r, t] = NEG if t < 128r + p else 0
            maskk = const.tile([128, 4, T], FP16)
            nc.gpsimd.memset(maskk, 0.0)
            for r in range(4):
                nc.gpsimd.affine_select(
                    out=maskk[:, r, :], in_=maskk[:, r, :],
                    compare_op=mybir.AluOpType.is_ge,
                    fill=NEG, base=-(128 * r),
                    channel_multiplier=-1, pattern=[[1, T]],
                )
            ones_c = const.tile([128, 1], FP16)
            nc.gpsimd.memset(ones_c, 1.0)
            warm = const.tile([1, 1], FP32)
            nc.scalar.activation(warm, identity[:1, :1], Exp)

            # ---- persistent SBUF ----
            woT_s = persist.tile([128, D], FP16)
            qT = persist.tile([128, TOK], FP16, tag="qT")
            oT = persist.tile([128, TOK], FP16, tag="oT")
            w_s = {}
            for name, w in (("q", wq), ("k", wk), ("v", wv)):
                w_s[name] = persist.tile(
                    [128, D // 128, 128], FP16, tag=f"w{name}", name=f"w{name}"
                )
            kT_b = [
                persist.tile([128, L], FP16, tag=f"kT{b}", name=f"kT{b}")
                for b in range(B)
            ]
            va_b = [
                persist.tile([128, HPC, NCH, HD + 1], FP16, tag=f"va{b}",
                             name=f"va{b}")
                for b in range(B)
            ]

            xT_r = xT.rearrange("(ko p) t -> p ko t", p=128)

            def dma_cache(b):
                nc.sync.dma_start(kT_b[b][:, :PAST], kTp[b, :, :])
                nc.sync.dma_start(va_b[b][:, :, :PCH, :], vp[b, :, :, :, :])
                # ones column for the 4 new-v chunks
                nc.vector.tensor_copy(
                    va_b[b][:, :, PCH:, HD],
                    ones_c[:, :, None].to_broadcast([128, HPC, NCH - PCH]),
                )

            def dma_x(b, xT_s=None):
                if xT_s is None:
                    xT_s = xs.tile([128, D // 128, 512], FP16, tag="xT")
                half = D // 256
                nc.sync.dma_start(xT_s[:, :half, :], xT_r[:, :half, bass.ts(b, 512)])
                nc.sync.dma_start(xT_s[:, half:, :], xT_r[:, half:, bass.ts(b, 512)])
                return xT_s

            def proj_qk(b, xT_s, name):
                dst = qT[:, bass.ts(b, T)] if name == "q" else kT_b[b][:, PAST:]
                ps = flexp.tile([128, 512], FP32, tag="flex")
                for ko in range(D // 128):
                    nc.tensor.matmul(
                        ps, lhsT=w_s[name][:, ko, :], rhs=xT_s[:, ko, :],
                        start=(ko == 0), stop=(ko == D // 128 - 1),
                    )
                nc.vector.tensor_copy(dst, ps)

            def proj_v(b, xT_s, tt):
                ps = flexp.tile([128, 512], FP32, tag="flex")
                for ko in range(D // 128):
                    nc.tensor.matmul(
                        ps[:, :128],
                        lhsT=xT_s[:, ko, bass.ts(tt, 128)],
                        rhs=w_s["v"][:, ko, :],
                        start=(ko == 0), stop=(ko == D // 128 - 1),
                    )
                for h in range(HPC):
                    nc.vector.tensor_copy(
                        va_b[b][:, h, PCH + tt, :HD],
                        ps[:, h * HD:(h + 1) * HD],
                    )

            def proj_pieces(b, xT_s):
                return [
                    lambda: proj_qk(b, xT_s, "q"),
                    lambda: proj_qk(b, xT_s, "k"),
                ] + [
                    (lambda tt: lambda: proj_v(b, xT_s, tt))(tt)
                    for tt in range(T // 128)
                ]

            def proj(b, xT_s):
                for piece in proj_pieces(b, xT_s):
                    piece()

            # ---- phase A: caches + projections for b0/b1 ----
            nc.sync.dma_start(w_s["q"], wq[:, :, :])
            xT_s0 = dma_x(0)
            nc.sync.dma_start(w_s["k"], wk[:, :, :])
            nc.sync.dma_start(w_s["v"], wv[:, :, :])
            dma_cache(0)
            xT_s1 = dma_x(1)
            dma_cache(1)
            proj(0, xT_s0)
            proj(1, xT_s1)
            nc.sync.dma_start(woT_s, woT[:, :])

            # ---- phase B: attention, both heads' chunk streams interleaved ----
            # One score chunk per 1-bank psum tile (ring of 4); each chunk's
            # softmax exp is ONE instruction on ONE engine (PSUM dep tracking
            # is bank-granular, so any split of a tile across engines would
            # serialize them). Chunks go 2:1 to ScalarE (true exp) : DVE
            # (Schraudolph); AV matmuls are deferred several slots so they
            # never stall PE's score stream.
            def attn(b, filler):
                bsl = bass.ts(b, T)
                kT = kT_b[b]
                va = va_b[b]
                accs = [
                    accp.tile([128, 512], FP32, tag="acc", name=f"acc{b}_{h}")
                    for h in range(HPC)
                ]

                def av(c, h, pT16):
                    qt0 = max(0, c - PCH)  # first query tile this chunk sees
                    for qt in range(qt0, 4):
                        # One accumulation group for the whole bank: HW (like
                        # the sim) zeroes the full 2KB zero-region on the first
                        # start=True and lazily zero-fills each byte's first
                        # write, so all four qt sub-ranges share the group.
                        nc.tensor.matmul(
                            accs[h][:, qt * 65:qt * 65 + 65],
                            lhsT=pT16[:, bass.ts(qt, 128)],
                            rhs=va[:, h, c, :],
                            start=(c == 0 and qt == 0),
                            stop=(c == NCH - 1 and qt == 3),
                            skip_group_check=True,
                        )

                pend = []
                slot = 0
                for c in range(NCH):
                    for h in range(HPC):
                        hsl = slice(h * HD, (h + 1) * HD)
                        off = max(0, (c - PCH) * 128)
                        masked = c >= PCH
                        S = scp.tile([128, 512], FP32, tag="sc")
                        nc.tensor.matmul(
                            S[:, off:],
                            lhsT=kT[hsl, bass.ts(c, 128)],
                            rhs=qT[hsl, bsl][:, off:],
                            start=True, stop=not masked,
                        )
                        if masked:
                            nc.tensor.matmul(
                                S[:, off:],
                                lhsT=id16,
                                rhs=maskk[:, c - PCH, off:],
                                start=False, stop=True,
                                skip_group_check=True,
                            )
                        if len(pend) >= 7:
                            av(*pend.pop(0))
                        if slot % 16 in (2, 5, 8, 11, 14):  # DVE schraudolph exp
                            pTd = ptdp.tile([128, 512], I16, tag="pTd")
                            nc.vector.tensor_scalar(
                                pTd[:, off:], S[:, off:], SCH_A, SCH_B,
                                op0=mult, op1=add,
                            )
                            pT16 = pTd.bitcast(FP16)
                        else:  # ScalarE true exp
                            pT16 = ptap.tile([128, 512], FP16, tag="pTa")
                            nc.scalar.activation(
                                pT16[:, off:], S[:, off:], Exp, scale=SCALE
                            )
                        pend.append((c, h, pT16))
                        if filler and slot % 6 == 3:
                            filler.pop(0)()
                        slot += 1
                for pp in pend:
                    av(*pp)
                # divide (denominator in col 64 of each qt block), transpose to oT
                for h in range(HPC):
                    hsl = slice(h * HD, (h + 1) * HD)
                    for qt in range(4):
                        a = accs[h][:, qt * 65:qt * 65 + 65]
                        r = ottp.tile([128, 1], FP32, tag="r")
                        nc.vector.reciprocal(r, a[:, 64:65])
                        ot = ottp.tile([128, 64], FP16, tag="ott")
                        nc.vector.tensor_scalar(ot, a[:, :64], r, None, op0=mult)
                        tp = flexp.tile([64, 128], FP16, tag="flex")
                        nc.tensor.transpose(tp, ot, id16)
                        nc.vector.tensor_copy(
                            oT[hsl, b * T + qt * 128:b * T + (qt + 1) * 128], tp
                        )
                while filler:
                    filler.pop(0)()

            def o_proj_piece(b, tt):
                out_r = out[bass.ts(b, T), :].rearrange("(tt p) d -> p tt d", p=128)
                tsl = slice(b * T + tt * 128, b * T + (tt + 1) * 128)
                ost = ostp.tile([128, D], FP16, tag="ost")
                for nh in range(2):
                    ps = flexp.tile([128, 512], FP32, tag="flex")
                    nc.tensor.matmul(
                        ps, lhsT=oT[:, tsl], rhs=woT_s[:, bass.ts(nh, 512)],
                        start=True, stop=True,
                    )
                    nc.vector.tensor_copy(ost[:, bass.ts(nh, 512)], ps)
                nc.sync.dma_start(out_r[:, tt, :], ost)

            def o_proj_pieces(b):
                return [
                    (lambda tt: lambda: o_proj_piece(b, tt))(tt)
                    for tt in range(T // 128)
                ]

            for b in range(B):
                filler = []
                if b + 2 < B:
                    xT_s = dma_x(b + 2)
                    dma_cache(b + 2)
                    filler += proj_pieces(b + 2, xT_s)
                if b > 0:
                    filler += o_proj_pieces(b - 1)
                attn(b, filler)
            for piece in o_proj_pieces(B - 1):
                piece()

    nc.compile()
    return nc


def _prep(x, k_prev, v_prev, Wq, Wk, Wv, Wo):
    """Host-side shard + fp16 layout marshalling."""
    f = np.float32
    x2 = np.ascontiguousarray(np.asarray(x, f).reshape(TOK, D))
    xT = np.ascontiguousarray(x2.T).astype(F16NP)
    k_prev = np.asarray(k_prev, f)
    v_prev = np.asarray(v_prev, f)
    Wq, Wk, Wv, Wo = (np.asarray(w, f) for w in (Wq, Wk, Wv, Wo))

    def wpack(Wrows):  # [128, D] -> [128dp, ko, 128m]: w[dp,ko,m] = W[m, 128ko+dp]
        return np.ascontiguousarray(
            Wrows.T.reshape(D // 128, 128, 128).transpose(1, 0, 2)
        ).astype(F16NP)

    in_maps = []
    for c in range(NCORES):
        rows = slice(128 * c, 128 * (c + 1))
        hsl = slice(HPC * c, HPC * (c + 1))
        kT = np.ascontiguousarray(
            k_prev[:, hsl, :, :].transpose(0, 1, 3, 2)
        ).reshape(B, 128, PAST).astype(F16NP)
        vpk = np.empty((B, 128, HPC, PCH, HD + 1), F16NP)
        vpk[..., :HD] = v_prev[:, hsl, :, :].reshape(
            B, HPC, PCH, 128, HD
        ).transpose(0, 3, 1, 2, 4).astype(F16NP)
        vpk[..., HD] = 1.0
        in_maps.append(
            {
                "xT": xT,
                "wq": wpack(Wq[rows, :]),
                "wk": wpack(Wk[rows, :]),
                "wv": wpack(Wv[rows, :]),
                "woT": np.ascontiguousarray(Wo[:, rows].T).astype(F16NP),
                "kTp": kT,
                "vp": np.ascontiguousarray(vpk),
            }
        )
    return in_maps


def kernel(x, k_prev, v_prev, Wq, Wk, Wv, Wo):
    if "nc" not in _cache:
        _cache["nc"] = _build()
    nc = _cache["nc"]
    in_maps = _prep(x, k_prev, v_prev, Wq, Wk, Wv, Wo)
    res = run_bass_kernel_spmd(nc, in_maps, core_ids=list(range(NCORES)))
    acc = np.zeros((TOK, D), np.float64)
    for r in res.results:
        acc += r["out"]
    return acc.astype(np.float32).reshape(B, T, D)


# revision 29
# speedup vs baseline: 1.3098x; 1.0057x over previous
"""Multi-head attention with KV cache, sharded over 8 NeuronCores by head.

Problem (hardcoded shapes):
  x       [4, 512, 1024]      hidden states (B, T, D)
  k_prev  [4, 16, 3584, 64]   KV cache (B, H, PAST, HD)
  v_prev  [4, 16, 3584, 64]
  Wq/Wk/Wv/Wo [1024, 1024]    projection weights (torch Linear: y = x @ W.T)

Sharding: 16 heads / 8 cores = 2 heads per core (data stays full along batch).
Each core computes q/k/v projections for its 2 heads (column-parallel),
full attention for its heads, and a column-parallel o_proj partial
[2048, 1024] in fp16; the host sums the 8 partials (the o_proj all-reduce).

Device algorithm per core (fp16 matmul operands, fp32 PSUM accumulate,
measured end-to-end rel err ~8e-3 incl. the partial Schraudolph softmax):
  - q/k projections: W_slice @ x^T on PE, contracting D; evicted fp16 into
    qT [128, TOK] and per-batch k caches kT_b [128, L] (cache DMA'd fp16).
  - v projection computed PRE-TRANSPOSED (out[token, hd] per 128-token tile)
    directly into the va value cache [128keys, 2h, 32chunk, 65] whose 65th
    column is 1.0 (softmax denominator rides the AV matmul).
  - scores^T[key, q] = k @ q^T per 128-key chunk (K=HD=64), one chunk per
    1-bank PSUM tile on a ring of 4; both heads' chunk streams interleave so
    two softmax chains are always in flight.  Causal mask on the 4 newest
    chunks accumulates an fp16 identity @ mask matmul into the same group
    (dtype must match the scores matmul: a mid-group dtype or perf-mode
    switch faults the PE).
  - softmax: one exp instruction per chunk on ONE engine (PSUM dependency
    tracking is bank-granular; splitting a tile across engines serializes
    them).  ~11/16 of chunks use true exp on ScalarE (scale=1/sqrt(HD)
    folded in; scores are O(1), no max subtraction), 5/16 use a one-op
    bias-corrected Schraudolph exp on DVE (rint to int16, bitcast fp16,
    ~1.8% rms ripple that largely cancels in the softmax ratio).
  - AV TRANSPOSED: per (chunk, 128-query tile): acc[q, 0:65] += pT_tile^T @
    [v|1] -- all four query tiles accumulate in ONE psum bank as a single
    accumulation group (HW zeroes the 2KB zero-region on the first
    start=True and lazily zero-fills each byte's first write), N=65 per
    matmul so PE cost is half of the straight orientation.  AV matmuls are
    deferred 7 slots so PE's score stream never stalls on exp.  Divide is a
    per-partition reciprocal+multiply (denominator in column 64), then a
    PE transpose (fp16) back to oT [hd, tok].
  - o_proj column-parallel per batch; fp16 partial [2048, 1024] written out.
  Projections for batch b+2 and o_proj for batch b-1 are emitted as filler
  pieces inside batch b's attention loop to fill PE gaps.
"""

import numpy as np

import concourse.bass as bass
import concourse.mybir as mybir
import concourse.tile as tile
from concourse import bacc
from concourse.bass_utils import run_bass_kernel_spmd
from concourse.masks import make_identity

B, T, D = 4, 512, 1024
H, HD = 16, 64
PAST = 3584
L = PAST + T            # 4096 == MAX_CACHE, nothing is trimmed
SCALE = float(1.0 / np.sqrt(HD))
NCORES = 8
HPC = H // NCORES       # heads per core = 2
TOK = B * T             # 2048
NCH = L // 128          # 32 key chunks per (b, h)

PCH = PAST // 128       # 28 chunks from the cache
NPAIR = NCH // 2        # 16 chunk pairs (one exp instruction each)
FP32 = mybir.dt.float32
FP16 = mybir.dt.float16
I16 = mybir.dt.int16
NEG = -256.0            # mask added in psum units; exp(-256/8) == 0 in fp16
F16NP = np.float16

# Every third (chunk, head) slot computes softmax exp on DVE via a
# bias-corrected Schraudolph approximation (int16 rint -> bitcast fp16,
# ~1.8% rms multiplicative ripple); the rest use true exp on ScalarE.
# At key-fraction 1/3 this adds ~8e-3 end-to-end rel err (gate is 2e-2).
SCH_A = float((1024.0 / np.log(2.0)) * SCALE)
SCH_B = 15360.0 - 59.6

_cache = {}


def _build():
    nc = bacc.Bacc(None, target_bir_lowering=False)

    xT = nc.dram_tensor("xT", [D, TOK], FP16, kind="ExternalInput")
    wq = nc.dram_tensor("wq", [128, D // 128, 128], FP16, kind="ExternalInput")
    wk = nc.dram_tensor("wk", [128, D // 128, 128], FP16, kind="ExternalInput")
    wv = nc.dram_tensor("wv", [128, D // 128, 128], FP16, kind="ExternalInput")
    woT = nc.dram_tensor("woT", [128, D], FP16, kind="ExternalInput")
    kTp = nc.dram_tensor("kTp", [B, 128, PAST], FP16, kind="ExternalInput")
    vp = nc.dram_tensor("vp", [B, 128, HPC, PCH, HD + 1], FP16, kind="ExternalInput")
    out = nc.dram_tensor("out", [TOK, D], FP16, kind="ExternalOutput")

    Exp = mybir.ActivationFunctionType.Exp
    mult = mybir.AluOpType.mult
    add = mybir.AluOpType.add

    with tile.TileContext(nc) as tc:
        with (
            tc.tile_pool(name="const", bufs=1) as const,
            tc.tile_pool(name="persist", bufs=1) as persist,
            tc.tile_pool(name="xs", bufs=2) as xs,
            tc.tile_pool(name="pta", bufs=9) as ptap,
            tc.tile_pool(name="ptd", bufs=6) as ptdp,
            tc.tile_pool(name="ott", bufs=4) as ottp,
            tc.tile_pool(name="ost", bufs=3) as ostp,
            tc.tile_pool(name="sc_ps", bufs=5, space="PSUM") as scp,
            tc.tile_pool(name="acc_ps", bufs=2, space="PSUM") as accp,
            tc.tile_pool(name="flex_ps", bufs=1, space="PSUM") as flexp,
        ):
            # ---- constants ----
            identity = const.tile([128, 128], FP32)
            make_identity(nc, identity)
            id16 = const.tile([128, 128], FP16)
            nc.vector.tensor_copy(id16, identity)
            # fp16 causal masks, applied via an identity matmul accumulated
            # into the scores group (dtype must match the scores matmul: a
            # mid-group dtype/perf-mode switch faults the PE).
            # maskk[p, r, t] = NEG if t < 128r + p else 0
            maskk = const.tile([128, 4, T], FP16)
            nc.gpsimd.memset(maskk, 0.0)
            for r in range(4):
                nc.gpsimd.affine_select(
                    out=maskk[:, r, :], in_=maskk[:, r, :],
                    compare_op=mybir.AluOpType.is_ge,
                    fill=NEG, base=-(128 * r),
                    channel_multiplier=-1, pattern=[[1, T]],
                )
            ones_c = const.tile([128, 1], FP16)
            nc.gpsimd.memset(ones_c, 1.0)
            warm = const.tile([1, 1], FP32)
            nc.scalar.activation(warm, identity[:1, :1], Exp)

            # ---- persistent SBUF ----
            woT_s = persist.tile([128, D], FP16)
            qT = persist.tile([128, TOK], FP16, tag="qT")
            oT = persist.tile([128, TOK], FP16, tag="oT")
            w_s = {}
            for name, w in (("q", wq), ("k", wk), ("v", wv)):
                w_s[name] = persist.tile(
                    [128, D // 128, 128], FP16, tag=f"w{name}", name=f"w{name}"
                )
            kT_b = [
                persist.tile([128, L], FP16, tag=f"kT{b}", name=f"kT{b}")
                for b in range(B)
            ]
            va_b = [
                persist.tile([128, HPC, NCH, HD + 1], FP16, tag=f"va{b}",
                             name=f"va{b}")
                for b in range(B)
            ]

            xT_r = xT.rearrange("(ko p) t -> p ko t", p=128)

            def dma_cache(b):
                nc.sync.dma_start(kT_b[b][:, :PAST], kTp[b, :, :])
                nc.sync.dma_start(va_b[b][:, :, :PCH, :], vp[b, :, :, :, :])
                # ones column for the 4 new-v chunks
                nc.vector.tensor_copy(
                    va_b[b][:, :, PCH:, HD],
                    ones_c[:, :, None].to_broadcast([128, HPC, NCH - PCH]),
                )

            def dma_x(b, xT_s=None):
                if xT_s is None:
                    xT_s = xs.tile([128, D // 128, 512], FP16, tag="xT")
                half = D // 256
                nc.sync.dma_start(xT_s[:, :half, :], xT_r[:, :half, bass.ts(b, 512)])
                nc.sync.dma_start(xT_s[:, half:, :], xT_r[:, half:, bass.ts(b, 512)])
                return xT_s

            def proj_qk(b, xT_s, name):
                dst = qT[:, bass.ts(b, T)] if name == "q" else kT_b[b][:, PAST:]
                ps = flexp.tile([128, 512], FP32, tag="flex")
                for ko in range(D // 128):
                    nc.tensor.matmul(
                        ps, lhsT=w_s[name][:, ko, :], rhs=xT_s[:, ko, :],
                        start=(ko == 0), stop=(ko == D // 128 - 1),
                    )
                nc.vector.tensor_copy(dst, ps)

            def proj_v(b, xT_s, tt):
                ps = flexp.tile([128, 512], FP32, tag="flex")
                for ko in range(D // 128):
                    nc.tensor.matmul(
                        ps[:, :128],
                        lhsT=xT_s[:, ko, bass.ts(tt, 128)],
                        rhs=w_s["v"][:, ko, :],
                        start=(ko == 0), stop=(ko == D // 128 - 1),
                    )
                for h in range(HPC):
                    nc.vector.tensor_copy(
                        va_b[b][:, h, PCH + tt, :HD],
                        ps[:, h * HD:(h + 1) * HD],
                    )

            def proj_pieces(b, xT_s):
                return [
                    lambda: proj_qk(b, xT_s, "q"),
                    lambda: proj_qk(b, xT_s, "k"),
                ] + [
                    (lambda tt: lambda: proj_v(b, xT_s, tt))(tt)
                    for tt in range(T // 128)
                ]

            def proj(b, xT_s):
                for piece in proj_pieces(b, xT_s):
                    piece()

            # ---- phase A: caches + projections for b0/b1 ----
            nc.sync.dma_start(w_s["q"], wq[:, :, :])
            xT_s0 = dma_x(0)
            nc.sync.dma_start(w_s["k"], wk[:, :, :])
            nc.sync.dma_start(w_s["v"], wv[:, :, :])
            dma_cache(0)
            xT_s1 = dma_x(1)
            dma_cache(1)
            proj(0, xT_s0)
            proj(1, xT_s1)
            nc.sync.dma_start(woT_s, woT[:, :])

            # ---- phase B: attention, both heads' chunk streams interleaved ----
            # One score chunk per 1-bank psum tile (ring of 4); each chunk's
            # softmax exp is ONE instruction on ONE engine (PSUM dep tracking
            # is bank-granular, so any split of a tile across engines would
            # serialize them). Chunks go 2:1 to ScalarE (true exp) : DVE
            # (Schraudolph); AV matmuls are deferred several slots so they
            # never stall PE's score stream.
            def attn(b, filler):
                bsl = bass.ts(b, T)
                kT = kT_b[b]
                va = va_b[b]
                accs = [
                    accp.tile([128, 512], FP32, tag="acc", name=f"acc{b}_{h}")
                    for h in range(HPC)
                ]

                def av(c, h, pT16):
                    qt0 = max(0, c - PCH)  # first query tile this chunk sees
                    for qt in range(qt0, 4):
                        # One accumulation group for the whole bank: HW (like
                        # the sim) zeroes the full 2KB zero-region on the first
                        # start=True and lazily zero-fills each byte's first
                        # write, so all four qt sub-ranges share the group.
                        nc.tensor.matmul(
                            accs[h][:, qt * 65:qt * 65 + 65],
                            lhsT=pT16[:, bass.ts(qt, 128)],
                            rhs=va[:, h, c, :],
                            start=(c == 0 and qt == 0),
                            stop=(c == NCH - 1 and qt == 3),
                            skip_group_check=True,
                        )

                pend = []
                slot = 0
                for c in range(NCH):
                    for h in range(HPC):
                        hsl = slice(h * HD, (h + 1) * HD)
                        off = max(0, (c - PCH) * 128)
                        masked = c >= PCH
                        S = scp.tile([128, 512], FP32, tag="sc")
                        nc.tensor.matmul(
                            S[:, off:],
                            lhsT=kT[hsl, bass.ts(c, 128)],
                            rhs=qT[hsl, bsl][:, off:],
                            start=True, stop=not masked,
                        )
                        if masked:
                            nc.tensor.matmul(
                                S[:, off:],
                                lhsT=id16,
                                rhs=maskk[:, c - PCH, off:],
                                start=False, stop=True,
                                skip_group_check=True,
                            )
                        if len(pend) >= 9:
                            av(*pend.pop(0))
                        if slot % 16 in (2, 5, 8, 11, 14):  # DVE schraudolph exp
                            pTd = ptdp.tile([128, 512], I16, tag="pTd")
                            nc.vector.tensor_scalar(
                                pTd[:, off:], S[:, off:], SCH_A, SCH_B,
                                op0=mult, op1=add,
                            )
                            pT16 = pTd.bitcast(FP16)
                        else:  # ScalarE true exp
                            pT16 = ptap.tile([128, 512], FP16, tag="pTa")
                            nc.scalar.activation(
                                pT16[:, off:], S[:, off:], Exp, scale=SCALE
                            )
                        pend.append((c, h, pT16))
                        if filler and slot % 6 == 3:
                            filler.pop(0)()
                        slot += 1
                for pp in pend:
                    av(*pp)
                # divide (denominator in col 64 of each qt block), transpose to oT
                for h in range(HPC):
                    hsl = slice(h * HD, (h + 1) * HD)
                    for qt in range(4):
                        a = accs[h][:, qt * 65:qt * 65 + 65]
                        r = ottp.tile([128, 1], FP32, tag="r")
                        nc.vector.reciprocal(r, a[:, 64:65])
                        ot = ottp.tile([128, 64], FP16, tag="ott")
                        nc.vector.tensor_scalar(ot, a[:, :64], r, None, op0=mult)
                        tp = flexp.tile([64, 128], FP16, tag="flex")
                        nc.tensor.transpose(tp, ot, id16)
                        nc.vector.tensor_copy(
                            oT[hsl, b * T + qt * 128:b * T + (qt + 1) * 128], tp
                        )
                while filler:
                    filler.pop(0)()

            def o_proj_piece(b, tt):
                out_r = out[bass.ts(b, T), :].rearrange("(tt p) d -> p tt d", p=128)
                tsl = slice(b * T + tt * 128, b * T + (tt + 1) * 128)
                ost = ostp.tile([128, D], FP16, tag="ost")
                for nh in range(2):
                    ps = flexp.tile([128, 512], FP32, tag="flex")
                    nc.tensor.matmul(
                        ps, lhsT=oT[:, tsl], rhs=woT_s[:, bass.ts(nh, 512)],
                        start=True, stop=True,
                    )
                    nc.vector.tensor_copy(ost[:, bass.ts(nh, 512)], ps)
                nc.sync.dma_start(out_r[:, tt, :], ost)

            def o_proj_pieces(b):
                return [
                    (lambda tt: lambda: o_proj_piece(b, tt))(tt)
                    for tt in range(T // 128)
                ]

            for b in range(B):
                filler = []
                if b + 2 < B:
                    xT_s = dma_x(b + 2)
                    dma_cache(b + 2)
                    filler += proj_pieces(b + 2, xT_s)
                if b > 0:
                    filler += o_proj_pieces(b - 1)
                attn(b, filler)
            for piece in o_proj_pieces(B - 1):
                piece()

    nc.compile()
    return nc


def _prep(x, k_prev, v_prev, Wq, Wk, Wv, Wo):
    """Host-side shard + fp16 layout marshalling."""
    f = np.float32
    x2 = np.ascontiguousarray(np.asarray(x, f).reshape(TOK, D))
    xT = np.ascontiguousarray(x2.T).astype(F16NP)
    k_prev = np.asarray(k_prev, f)
    v_prev = np.asarray(v_prev, f)
    Wq, Wk, Wv, Wo = (np.asarray(w, f) for w in (Wq, Wk, Wv, Wo))

    def wpack(Wrows):  # [128, D] -> [128dp, ko, 128m]: w[dp,ko,m] = W[m, 128ko+dp]
        return np.ascontiguousarray(
            Wrows.T.reshape(D // 128, 128, 128).transpose(1, 0, 2)
        ).astype(F16NP)

    in_maps = []
    for c in range(NCORES):
        rows = slice(128 * c, 128 * (c + 1))
        hsl = slice(HPC * c, HPC * (c + 1))
        kT = np.ascontiguousarray(
            k_prev[:, hsl, :, :].transpose(0, 1, 3, 2)
        ).reshape(B, 128, PAST).astype(F16NP)
        vpk = np.empty((B, 128, HPC, PCH, HD + 1), F16NP)
        vpk[..., :HD] = v_prev[:, hsl, :, :].reshape(
            B, HPC, PCH, 128, HD
        ).transpose(0, 3, 1, 2, 4).astype(F16NP)
        vpk[..., HD] = 1.0
        in_maps.append(
            {
                "xT": xT,
                "wq": wpack(Wq[rows, :]),
                "wk": wpack(Wk[rows, :]),
                "wv": wpack(Wv[rows, :]),
                "woT": np.ascontiguousarray(Wo[:, rows].T).astype(F16NP),
                "kTp": kT,
                "vp": np.ascontiguousarray(vpk),
            }
        )
    return in_maps


def kernel(x, k_prev, v_prev, Wq, Wk, Wv, Wo):
    if "nc" not in _cache:
        _cache["nc"] = _build()
    nc = _cache["nc"]
    in_maps = _prep(x, k_prev, v_prev, Wq, Wk, Wv, Wo)
    res = run_bass_kernel_spmd(nc, in_maps, core_ids=list(range(NCORES)))
    acc = np.zeros((TOK, D), np.float64)
    for r in res.results:
        acc += r["out"]
    return acc.astype(np.float32).reshape(B, T, D)


# revision 32
# speedup vs baseline: 1.3100x; 1.0001x over previous
"""Multi-head attention with KV cache, sharded over 8 NeuronCores by head.

Problem (hardcoded shapes):
  x       [4, 512, 1024]      hidden states (B, T, D)
  k_prev  [4, 16, 3584, 64]   KV cache (B, H, PAST, HD)
  v_prev  [4, 16, 3584, 64]
  Wq/Wk/Wv/Wo [1024, 1024]    projection weights (torch Linear: y = x @ W.T)

Sharding: 16 heads / 8 cores = 2 heads per core (data stays full along batch).
Each core computes q/k/v projections for its 2 heads (column-parallel),
full attention for its heads, and a column-parallel o_proj partial
[2048, 1024] in fp16; the host sums the 8 partials (the o_proj all-reduce).

Device algorithm per core (fp16 matmul operands, fp32 PSUM accumulate,
measured end-to-end rel err ~8e-3 incl. the partial Schraudolph softmax):
  - q/k projections: W_slice @ x^T on PE, contracting D; evicted fp16 into
    qT [128, TOK] and per-batch k caches kT_b [128, L] (cache DMA'd fp16).
  - v projection computed PRE-TRANSPOSED (out[token, hd] per 128-token tile)
    directly into the va value cache [128keys, 2h, 32chunk, 65] whose 65th
    column is 1.0 (softmax denominator rides the AV matmul).
  - scores^T[key, q] = k @ q^T per 128-key chunk (K=HD=64), one chunk per
    1-bank PSUM tile on a ring of 4; both heads' chunk streams interleave so
    two softmax chains are always in flight.  Causal mask on the 4 newest
    chunks accumulates an fp16 identity @ mask matmul into the same group
    (dtype must match the scores matmul: a mid-group dtype or perf-mode
    switch faults the PE).
  - softmax: one exp instruction per chunk on ONE engine (PSUM dependency
    tracking is bank-granular; splitting a tile across engines serializes
    them).  ~11/16 of chunks use true exp on ScalarE (scale=1/sqrt(HD)
    folded in; scores are O(1), no max subtraction), 5/16 use a one-op
    bias-corrected Schraudolph exp on DVE (rint to int16, bitcast fp16,
    ~1.8% rms ripple that largely cancels in the softmax ratio).
  - AV TRANSPOSED: per (chunk, 128-query tile): acc[q, 0:65] += pT_tile^T @
    [v|1] -- all four query tiles accumulate in ONE psum bank as a single
    accumulation group (HW zeroes the 2KB zero-region on the first
    start=True and lazily zero-fills each byte's first write), N=65 per
    matmul so PE cost is half of the straight orientation.  AV matmuls are
    deferred 7 slots so PE's score stream never stalls on exp.  Divide is a
    per-partition reciprocal+multiply (denominator in column 64), then a
    PE transpose (fp16) back to oT [hd, tok].
  - o_proj column-parallel per batch; fp16 partial [2048, 1024] written out.
  Projections for batch b+2 and o_proj for batch b-1 are emitted as filler
  pieces inside batch b's attention loop to fill PE gaps.
"""

import numpy as np

import concourse.bass as bass
import concourse.mybir as mybir
import concourse.tile as tile
from concourse import bacc
from concourse.bass_utils import run_bass_kernel_spmd
from concourse.masks import make_identity

B, T, D = 4, 512, 1024
H, HD = 16, 64
PAST = 3584
L = PAST + T            # 4096 == MAX_CACHE, nothing is trimmed
SCALE = float(1.0 / np.sqrt(HD))
NCORES = 8
HPC = H // NCORES       # heads per core = 2
TOK = B * T             # 2048
NCH = L // 128          # 32 key chunks per (b, h)

PCH = PAST // 128       # 28 chunks from the cache
NPAIR = NCH // 2        # 16 chunk pairs (one exp instruction each)
FP32 = mybir.dt.float32
FP16 = mybir.dt.float16
I16 = mybir.dt.int16
NEG = -256.0            # mask added in psum units; exp(-256/8) == 0 in fp16
F16NP = np.float16

# Every third (chunk, head) slot computes softmax exp on DVE via a
# bias-corrected Schraudolph approximation (int16 rint -> bitcast fp16,
# ~1.8% rms multiplicative ripple); the rest use true exp on ScalarE.
# At key-fraction 1/3 this adds ~8e-3 end-to-end rel err (gate is 2e-2).
SCH_A = float((1024.0 / np.log(2.0)) * SCALE)
SCH_B = 15360.0 - 59.6

_cache = {}


def _build():
    nc = bacc.Bacc(None, target_bir_lowering=False)

    xT = nc.dram_tensor("xT", [D, TOK], FP16, kind="ExternalInput")
    wq = nc.dram_tensor("wq", [128, D // 128, 128], FP16, kind="ExternalInput")
    wk = nc.dram_tensor("wk", [128, D // 128, 128], FP16, kind="ExternalInput")
    wv = nc.dram_tensor("wv", [128, D // 128, 128], FP16, kind="ExternalInput")
    woT = nc.dram_tensor("woT", [128, D], FP16, kind="ExternalInput")
    kTp = nc.dram_tensor("kTp", [B, 128, PAST], FP16, kind="ExternalInput")
    vp = nc.dram_tensor("vp", [B, 128, HPC, PCH, HD + 1], FP16, kind="ExternalInput")
    out = nc.dram_tensor("out", [TOK, D], FP16, kind="ExternalOutput")

    Exp = mybir.ActivationFunctionType.Exp
    mult = mybir.AluOpType.mult
    add = mybir.AluOpType.add

    with tile.TileContext(nc) as tc:
        with (
            tc.tile_pool(name="const", bufs=1) as const,
            tc.tile_pool(name="persist", bufs=1) as persist,
            tc.tile_pool(name="xs", bufs=2) as xs,
            tc.tile_pool(name="pta", bufs=9) as ptap,
            tc.tile_pool(name="ptd", bufs=6) as ptdp,
            tc.tile_pool(name="ott", bufs=4) as ottp,
            tc.tile_pool(name="ost", bufs=3) as ostp,
            tc.tile_pool(name="sc_ps", bufs=4, space="PSUM") as scp,
            tc.tile_pool(name="acc_ps", bufs=2, space="PSUM") as accp,
            tc.tile_pool(name="flex_ps", bufs=2, space="PSUM") as flexp,
        ):
            # ---- constants ----
            identity = const.tile([128, 128], FP32)
            make_identity(nc, identity)
            id16 = const.tile([128, 128], FP16)
            nc.vector.tensor_copy(id16, identity)
            # fp16 causal masks, applied via an identity matmul accumulated
            # into the scores group (dtype must match the scores matmul: a
            # mid-group dtype/perf-mode switch faults the PE).
            # maskk[p, r, t] = NEG if t < 128r + p else 0
            maskk = const.tile([128, 4, T], FP16)
            nc.gpsimd.memset(maskk, 0.0)
            for r in range(4):
                nc.gpsimd.affine_select(
                    out=maskk[:, r, :], in_=maskk[:, r, :],
                    compare_op=mybir.AluOpType.is_ge,
                    fill=NEG, base=-(128 * r),
                    channel_multiplier=-1, pattern=[[1, T]],
                )
            ones_c = const.tile([128, 1], FP16)
            nc.gpsimd.memset(ones_c, 1.0)
            warm = const.tile([1, 1], FP32)
            nc.scalar.activation(warm, identity[:1, :1], Exp)

            # ---- persistent SBUF ----
            woT_s = persist.tile([128, D], FP16)
            qT = persist.tile([128, TOK], FP16, tag="qT")
            oT = persist.tile([128, TOK], FP16, tag="oT")
            w_s = {}
            for name, w in (("q", wq), ("k", wk), ("v", wv)):
                w_s[name] = persist.tile(
                    [128, D // 128, 128], FP16, tag=f"w{name}", name=f"w{name}"
                )
            kT_b = [
                persist.tile([128, L], FP16, tag=f"kT{b}", name=f"kT{b}")
                for b in range(B)
            ]
            va_b = [
                persist.tile([128, HPC, NCH, HD + 1], FP16, tag=f"va{b}",
                             name=f"va{b}")
                for b in range(B)
            ]

            xT_r = xT.rearrange("(ko p) t -> p ko t", p=128)

            def dma_cache(b):
                nc.sync.dma_start(kT_b[b][:, :PAST], kTp[b, :, :])
                nc.sync.dma_start(va_b[b][:, :, :PCH, :], vp[b, :, :, :, :])
                # ones column for the 4 new-v chunks
                nc.vector.tensor_copy(
                    va_b[b][:, :, PCH:, HD],
                    ones_c[:, :, None].to_broadcast([128, HPC, NCH - PCH]),
                )

            def dma_x(b, xT_s=None):
                if xT_s is None:
                    xT_s = xs.tile([128, D // 128, 512], FP16, tag="xT")
                half = D // 256
                nc.sync.dma_start(xT_s[:, :half, :], xT_r[:, :half, bass.ts(b, 512)])
                nc.sync.dma_start(xT_s[:, half:, :], xT_r[:, half:, bass.ts(b, 512)])
                return xT_s

            def proj_qk(b, xT_s, name):
                dst = qT[:, bass.ts(b, T)] if name == "q" else kT_b[b][:, PAST:]
                ps = flexp.tile([128, 512], FP32, tag="flex")
                for ko in range(D // 128):
                    nc.tensor.matmul(
                        ps, lhsT=w_s[name][:, ko, :], rhs=xT_s[:, ko, :],
                        start=(ko == 0), stop=(ko == D // 128 - 1),
                    )
                nc.vector.tensor_copy(dst, ps)

            def proj_v(b, xT_s, tt):
                ps = flexp.tile([128, 512], FP32, tag="flex")
                for ko in range(D // 128):
                    nc.tensor.matmul(
                        ps[:, :128],
                        lhsT=xT_s[:, ko, bass.ts(tt, 128)],
                        rhs=w_s["v"][:, ko, :],
                        start=(ko == 0), stop=(ko == D // 128 - 1),
                    )
                for h in range(HPC):
                    nc.vector.tensor_copy(
                        va_b[b][:, h, PCH + tt, :HD],
                        ps[:, h * HD:(h + 1) * HD],
                    )

            def proj_pieces(b, xT_s):
                return [
                    lambda: proj_qk(b, xT_s, "q"),
                    lambda: proj_qk(b, xT_s, "k"),
                ] + [
                    (lambda tt: lambda: proj_v(b, xT_s, tt))(tt)
                    for tt in range(T // 128)
                ]

            def proj(b, xT_s):
                for piece in proj_pieces(b, xT_s):
                    piece()

            # ---- phase A: caches + projections for b0/b1 ----
            nc.sync.dma_start(w_s["q"], wq[:, :, :])
            xT_s0 = dma_x(0)
            nc.sync.dma_start(w_s["k"], wk[:, :, :])
            nc.sync.dma_start(w_s["v"], wv[:, :, :])
            dma_cache(0)
            xT_s1 = dma_x(1)
            dma_cache(1)
            proj(0, xT_s0)
            proj(1, xT_s1)
            nc.sync.dma_start(woT_s, woT[:, :])

            # ---- phase B: attention, both heads' chunk streams interleaved ----
            # One score chunk per 1-bank psum tile (ring of 4); each chunk's
            # softmax exp is ONE instruction on ONE engine (PSUM dep tracking
            # is bank-granular, so any split of a tile across engines would
            # serialize them). Chunks go 2:1 to ScalarE (true exp) : DVE
            # (Schraudolph); AV matmuls are deferred several slots so they
            # never stall PE's score stream.
            def attn(b, filler):
                bsl = bass.ts(b, T)
                kT = kT_b[b]
                va = va_b[b]
                accs = [
                    accp.tile([128, 512], FP32, tag="acc", name=f"acc{b}_{h}")
                    for h in range(HPC)
                ]

                def av(c, h, pT16):
                    qt0 = max(0, c - PCH)  # first query tile this chunk sees
                    for qt in range(qt0, 4):
                        # One accumulation group for the whole bank: HW (like
                        # the sim) zeroes the full 2KB zero-region on the first
                        # start=True and lazily zero-fills each byte's first
                        # write, so all four qt sub-ranges share the group.
                        nc.tensor.matmul(
                            accs[h][:, qt * 65:qt * 65 + 65],
                            lhsT=pT16[:, bass.ts(qt, 128)],
                            rhs=va[:, h, c, :],
                            start=(c == 0 and qt == 0),
                            stop=(c == NCH - 1 and qt == 3),
                            skip_group_check=True,
                        )

                pend = []
                slot = 0
                for c in range(NCH):
                    for h in range(HPC):
                        hsl = slice(h * HD, (h + 1) * HD)
                        off = max(0, (c - PCH) * 128)
                        masked = c >= PCH
                        S = scp.tile([128, 512], FP32, tag="sc")
                        nc.tensor.matmul(
                            S[:, off:],
                            lhsT=kT[hsl, bass.ts(c, 128)],
                            rhs=qT[hsl, bsl][:, off:],
                            start=True, stop=not masked,
                        )
                        if masked:
                            nc.tensor.matmul(
                                S[:, off:],
                                lhsT=id16,
                                rhs=maskk[:, c - PCH, off:],
                                start=False, stop=True,
                                skip_group_check=True,
                            )
                        if len(pend) >= 9:
                            av(*pend.pop(0))
                        if slot % 16 in (2, 5, 8, 11, 14):  # DVE schraudolph exp
                            pTd = ptdp.tile([128, 512], I16, tag="pTd")
                            nc.vector.tensor_scalar(
                                pTd[:, off:], S[:, off:], SCH_A, SCH_B,
                                op0=mult, op1=add,
                            )
                            pT16 = pTd.bitcast(FP16)
                        else:  # ScalarE true exp
                            pT16 = ptap.tile([128, 512], FP16, tag="pTa")
                            nc.scalar.activation(
                                pT16[:, off:], S[:, off:], Exp, scale=SCALE
                            )
                        pend.append((c, h, pT16))
                        if filler and slot % 6 == 3:
                            filler.pop(0)()
                        slot += 1
                for pp in pend:
                    av(*pp)
                # divide (denominator in col 64 of each qt block), transpose to oT
                for h in range(HPC):
                    hsl = slice(h * HD, (h + 1) * HD)
                    for qt in range(4):
                        a = accs[h][:, qt * 65:qt * 65 + 65]
                        r = ottp.tile([128, 1], FP32, tag="r")
                        nc.vector.reciprocal(r, a[:, 64:65])
                        ot = ottp.tile([128, 64], FP16, tag="ott")
                        nc.vector.tensor_scalar(ot, a[:, :64], r, None, op0=mult)
                        tp = flexp.tile([64, 128], FP16, tag="flex")
                        nc.tensor.transpose(tp, ot, id16)
                        nc.vector.tensor_copy(
                            oT[hsl, b * T + qt * 128:b * T + (qt + 1) * 128], tp
                        )
                while filler:
                    filler.pop(0)()

            def o_proj_piece(b, tt):
                out_r = out[bass.ts(b, T), :].rearrange("(tt p) d -> p tt d", p=128)
                tsl = slice(b * T + tt * 128, b * T + (tt + 1) * 128)
                ost = ostp.tile([128, D], FP16, tag="ost")
                for nh in range(2):
                    ps = flexp.tile([128, 512], FP32, tag="flex")
                    nc.tensor.matmul(
                        ps, lhsT=oT[:, tsl], rhs=woT_s[:, bass.ts(nh, 512)],
                        start=True, stop=True,
                    )
                    nc.vector.tensor_copy(ost[:, bass.ts(nh, 512)], ps)
                nc.sync.dma_start(out_r[:, tt, :], ost)

            def o_proj_pieces(b):
                return [
                    (lambda tt: lambda: o_proj_piece(b, tt))(tt)
                    for tt in range(T // 128)
                ]

            for b in range(B):
                filler = []
                if b + 2 < B:
                    xT_s = dma_x(b + 2)
                    dma_cache(b + 2)
                    filler += proj_pieces(b + 2, xT_s)
                if b > 0:
                    filler += o_proj_pieces(b - 1)
                attn(b, filler)
            for piece in o_proj_pieces(B - 1):
                piece()

    nc.compile()
    return nc


def _prep(x, k_prev, v_prev, Wq, Wk, Wv, Wo):
    """Host-side shard + fp16 layout marshalling."""
    f = np.float32
    x2 = np.ascontiguousarray(np.asarray(x, f).reshape(TOK, D))
    xT = np.ascontiguousarray(x2.T).astype(F16NP)
    k_prev = np.asarray(k_prev, f)
    v_prev = np.asarray(v_prev, f)
    Wq, Wk, Wv, Wo = (np.asarray(w, f) for w in (Wq, Wk, Wv, Wo))

    def wpack(Wrows):  # [128, D] -> [128dp, ko, 128m]: w[dp,ko,m] = W[m, 128ko+dp]
        return np.ascontiguousarray(
            Wrows.T.reshape(D // 128, 128, 128).transpose(1, 0, 2)
        ).astype(F16NP)

    in_maps = []
    for c in range(NCORES):
        rows = slice(128 * c, 128 * (c + 1))
        hsl = slice(HPC * c, HPC * (c + 1))
        kT = np.ascontiguousarray(
            k_prev[:, hsl, :, :].transpose(0, 1, 3, 2)
        ).reshape(B, 128, PAST).astype(F16NP)
        vpk = np.empty((B, 128, HPC, PCH, HD + 1), F16NP)
        vpk[..., :HD] = v_prev[:, hsl, :, :].reshape(
            B, HPC, PCH, 128, HD
        ).transpose(0, 3, 1, 2, 4).astype(F16NP)
        vpk[..., HD] = 1.0
        in_maps.append(
            {
                "xT": xT,
                "wq": wpack(Wq[rows, :]),
                "wk": wpack(Wk[rows, :]),
                "wv": wpack(Wv[rows, :]),
                "woT": np.ascontiguousarray(Wo[:, rows].T).astype(F16NP),
                "kTp": kT,
                "vp": np.ascontiguousarray(vpk),
            }
        )
    return in_maps


def kernel(x, k_prev, v_prev, Wq, Wk, Wv, Wo):
    if "nc" not in _cache:
        _cache["nc"] = _build()
    nc = _cache["nc"]
    in_maps = _prep(x, k_prev, v_prev, Wq, Wk, Wv, Wo)
    res = run_bass_kernel_spmd(nc, in_maps, core_ids=list(range(NCORES)))
    acc = np.zeros((TOK, D), np.float64)
    for r in res.results:
        acc += r["out"]
    return acc.astype(np.float32).reshape(B, T, D)


# revision 33
# speedup vs baseline: 1.3117x; 1.0014x over previous
"""Multi-head attention with KV cache, sharded over 8 NeuronCores by head.

Problem (hardcoded shapes):
  x       [4, 512, 1024]      hidden states (B, T, D)
  k_prev  [4, 16, 3584, 64]   KV cache (B, H, PAST, HD)
  v_prev  [4, 16, 3584, 64]
  Wq/Wk/Wv/Wo [1024, 1024]    projection weights (torch Linear: y = x @ W.T)

Sharding: 16 heads / 8 cores = 2 heads per core (data stays full along batch).
Each core computes q/k/v projections for its 2 heads (column-parallel),
full attention for its heads, and a column-parallel o_proj partial
[2048, 1024] in fp16; the host sums the 8 partials (the o_proj all-reduce).

Device algorithm per core (fp16 matmul operands, fp32 PSUM accumulate,
measured end-to-end rel err ~8e-3 incl. the partial Schraudolph softmax):
  - q/k projections: W_slice @ x^T on PE, contracting D; evicted fp16 into
    qT [128, TOK] and per-batch k caches kT_b [128, L] (cache DMA'd fp16).
  - v projection computed PRE-TRANSPOSED (out[token, hd] per 128-token tile)
    directly into the va value cache [128keys, 2h, 32chunk, 65] whose 65th
    column is 1.0 (softmax denominator rides the AV matmul).
  - scores^T[key, q] = k @ q^T per 128-key chunk (K=HD=64), one chunk per
    1-bank PSUM tile on a ring of 4; both heads' chunk streams interleave so
    two softmax chains are always in flight.  Causal mask on the 4 newest
    chunks accumulates an fp16 identity @ mask matmul into the same group
    (dtype must match the scores matmul: a mid-group dtype or perf-mode
    switch faults the PE).
  - softmax: one exp instruction per chunk on ONE engine (PSUM dependency
    tracking is bank-granular; splitting a tile across engines serializes
    them).  ~11/16 of chunks use true exp on ScalarE (scale=1/sqrt(HD)
    folded in; scores are O(1), no max subtraction), 5/16 use a one-op
    bias-corrected Schraudolph exp on DVE (rint to int16, bitcast fp16,
    ~1.8% rms ripple that largely cancels in the softmax ratio).
  - AV TRANSPOSED: per (chunk, 128-query tile): acc[q, 0:65] += pT_tile^T @
    [v|1] -- all four query tiles accumulate in ONE psum bank as a single
    accumulation group (HW zeroes the 2KB zero-region on the first
    start=True and lazily zero-fills each byte's first write), N=65 per
    matmul so PE cost is half of the straight orientation.  AV matmuls are
    deferred 7 slots so PE's score stream never stalls on exp.  Divide is a
    per-partition reciprocal+multiply (denominator in column 64), then a
    PE transpose (fp16) back to oT [hd, tok].
  - o_proj column-parallel per batch; fp16 partial [2048, 1024] written out.
  Projections for batch b+2 and o_proj for batch b-1 are emitted as filler
  pieces inside batch b's attention loop to fill PE gaps.
"""

import numpy as np

import concourse.bass as bass
import concourse.mybir as mybir
import concourse.tile as tile
from concourse import bacc
from concourse.bass_utils import run_bass_kernel_spmd
from concourse.masks import make_identity

B, T, D = 4, 512, 1024
H, HD = 16, 64
PAST = 3584
L = PAST + T            # 4096 == MAX_CACHE, nothing is trimmed
SCALE = float(1.0 / np.sqrt(HD))
NCORES = 8
HPC = H // NCORES       # heads per core = 2
TOK = B * T             # 2048
NCH = L // 128          # 32 key chunks per (b, h)

PCH = PAST // 128       # 28 chunks from the cache
NPAIR = NCH // 2        # 16 chunk pairs (one exp instruction each)
FP32 = mybir.dt.float32
FP16 = mybir.dt.float16
I16 = mybir.dt.int16
NEG = -256.0            # mask added in psum units; exp(-256/8) == 0 in fp16
F16NP = np.float16

# Every third (chunk, head) slot computes softmax exp on DVE via a
# bias-corrected Schraudolph approximation (int16 rint -> bitcast fp16,
# ~1.8% rms multiplicative ripple); the rest use true exp on ScalarE.
# At key-fraction 1/3 this adds ~8e-3 end-to-end rel err (gate is 2e-2).
SCH_A = float((1024.0 / np.log(2.0)) * SCALE)
SCH_B = 15360.0 - 59.6

_cache = {}


def _build():
    nc = bacc.Bacc(None, target_bir_lowering=False)

    xT = nc.dram_tensor("xT", [D, TOK], FP16, kind="ExternalInput")
    wq = nc.dram_tensor("wq", [128, D // 128, 128], FP16, kind="ExternalInput")
    wk = nc.dram_tensor("wk", [128, D // 128, 128], FP16, kind="ExternalInput")
    wv = nc.dram_tensor("wv", [128, D // 128, 128], FP16, kind="ExternalInput")
    woT = nc.dram_tensor("woT", [128, D], FP16, kind="ExternalInput")
    kTp = nc.dram_tensor("kTp", [B, 128, PAST], FP16, kind="ExternalInput")
    vp = nc.dram_tensor("vp", [B, 128, HPC, PCH, HD + 1], FP16, kind="ExternalInput")
    out = nc.dram_tensor("out", [TOK, D], FP16, kind="ExternalOutput")

    Exp = mybir.ActivationFunctionType.Exp
    mult = mybir.AluOpType.mult
    add = mybir.AluOpType.add

    with tile.TileContext(nc) as tc:
        with (
            tc.tile_pool(name="const", bufs=1) as const,
            tc.tile_pool(name="persist", bufs=1) as persist,
            tc.tile_pool(name="xs", bufs=2) as xs,
            tc.tile_pool(name="pta", bufs=9) as ptap,
            tc.tile_pool(name="ptd", bufs=6) as ptdp,
            tc.tile_pool(name="ott", bufs=4) as ottp,
            tc.tile_pool(name="ost", bufs=3) as ostp,
            tc.tile_pool(name="sc_ps", bufs=4, space="PSUM") as scp,
            tc.tile_pool(name="acc_ps", bufs=2, space="PSUM") as accp,
            tc.tile_pool(name="flex_ps", bufs=2, space="PSUM") as flexp,
        ):
            # ---- constants ----
            identity = const.tile([128, 128], FP32)
            make_identity(nc, identity)
            id16 = const.tile([128, 128], FP16)
            nc.vector.tensor_copy(id16, identity)
            # fp16 causal masks, applied via an identity matmul accumulated
            # into the scores group (dtype must match the scores matmul: a
            # mid-group dtype/perf-mode switch faults the PE).
            # maskk[p, r, t] = NEG if t < 128r + p else 0
            maskk = const.tile([128, 4, T], FP16)
            nc.gpsimd.memset(maskk, 0.0)
            for r in range(4):
                nc.gpsimd.affine_select(
                    out=maskk[:, r, :], in_=maskk[:, r, :],
                    compare_op=mybir.AluOpType.is_ge,
                    fill=NEG, base=-(128 * r),
                    channel_multiplier=-1, pattern=[[1, T]],
                )
            ones_c = const.tile([128, 1], FP16)
            nc.gpsimd.memset(ones_c, 1.0)
            warm = const.tile([1, 1], FP32)
            nc.scalar.activation(warm, identity[:1, :1], Exp)

            # ---- persistent SBUF ----
            woT_s = persist.tile([128, D], FP16)
            qT = persist.tile([128, TOK], FP16, tag="qT")
            oT = persist.tile([128, TOK], FP16, tag="oT")
            w_s = {}
            for name, w in (("q", wq), ("k", wk), ("v", wv)):
                w_s[name] = persist.tile(
                    [128, D // 128, 128], FP16, tag=f"w{name}", name=f"w{name}"
                )
            kT_b = [
                persist.tile([128, L], FP16, tag=f"kT{b}", name=f"kT{b}")
                for b in range(B)
            ]
            va_b = [
                persist.tile([128, HPC, NCH, HD + 1], FP16, tag=f"va{b}",
                             name=f"va{b}")
                for b in range(B)
            ]

            xT_r = xT.rearrange("(ko p) t -> p ko t", p=128)

            def dma_cache(b):
                nc.sync.dma_start(kT_b[b][:, :PAST], kTp[b, :, :])
                nc.sync.dma_start(va_b[b][:, :, :PCH, :], vp[b, :, :, :, :])
                # ones column for the 4 new-v chunks
                nc.vector.tensor_copy(
                    va_b[b][:, :, PCH:, HD],
                    ones_c[:, :, None].to_broadcast([128, HPC, NCH - PCH]),
                )

            def dma_x(b, xT_s=None):
                if xT_s is None:
                    xT_s = xs.tile([128, D // 128, 512], FP16, tag="xT")
                half = D // 256
                nc.sync.dma_start(xT_s[:, :half, :], xT_r[:, :half, bass.ts(b, 512)])
                nc.sync.dma_start(xT_s[:, half:, :], xT_r[:, half:, bass.ts(b, 512)])
                return xT_s

            def proj_qk(b, xT_s, name):
                dst = qT[:, bass.ts(b, T)] if name == "q" else kT_b[b][:, PAST:]
                ps = flexp.tile([128, 512], FP32, tag="flex")
                for ko in range(D // 128):
                    nc.tensor.matmul(
                        ps, lhsT=w_s[name][:, ko, :], rhs=xT_s[:, ko, :],
                        start=(ko == 0), stop=(ko == D // 128 - 1),
                    )
                nc.vector.tensor_copy(dst, ps)

            def proj_v(b, xT_s, tt):
                ps = flexp.tile([128, 512], FP32, tag="flex")
                for ko in range(D // 128):
                    nc.tensor.matmul(
                        ps[:, :128],
                        lhsT=xT_s[:, ko, bass.ts(tt, 128)],
                        rhs=w_s["v"][:, ko, :],
                        start=(ko == 0), stop=(ko == D // 128 - 1),
                    )
                for h in range(HPC):
                    nc.vector.tensor_copy(
                        va_b[b][:, h, PCH + tt, :HD],
                        ps[:, h * HD:(h + 1) * HD],
                    )

            def proj_pieces(b, xT_s):
                return [
                    lambda: proj_qk(b, xT_s, "q"),
                    lambda: proj_qk(b, xT_s, "k"),
                ] + [
                    (lambda tt: lambda: proj_v(b, xT_s, tt))(tt)
                    for tt in range(T // 128)
                ]

            def proj(b, xT_s):
                for piece in proj_pieces(b, xT_s):
                    piece()

            # ---- phase A: caches + projections for b0/b1 ----
            nc.sync.dma_start(w_s["q"], wq[:, :, :])
            xT_s0 = dma_x(0)
            nc.sync.dma_start(w_s["k"], wk[:, :, :])
            nc.sync.dma_start(w_s["v"], wv[:, :, :])
            dma_cache(0)
            xT_s1 = dma_x(1)
            dma_cache(1)
            proj(0, xT_s0)
            proj(1, xT_s1)
            nc.sync.dma_start(woT_s, woT[:, :])

            # ---- phase B: attention, both heads' chunk streams interleaved ----
            # One score chunk per 1-bank psum tile (ring of 4); each chunk's
            # softmax exp is ONE instruction on ONE engine (PSUM dep tracking
            # is bank-granular, so any split of a tile across engines would
            # serialize them). Chunks go 2:1 to ScalarE (true exp) : DVE
            # (Schraudolph); AV matmuls are deferred several slots so they
            # never stall PE's score stream.
            def attn(b, filler):
                bsl = bass.ts(b, T)
                kT = kT_b[b]
                va = va_b[b]
                accs = [
                    accp.tile([128, 512], FP32, tag="acc", name=f"acc{b}_{h}")
                    for h in range(HPC)
                ]

                def av(c, h, pT16):
                    qt0 = max(0, c - PCH)  # first query tile this chunk sees
                    for qt in range(qt0, 4):
                        # One accumulation group for the whole bank: HW (like
                        # the sim) zeroes the full 2KB zero-region on the first
                        # start=True and lazily zero-fills each byte's first
                        # write, so all four qt sub-ranges share the group.
                        nc.tensor.matmul(
                            accs[h][:, qt * 65:qt * 65 + 65],
                            lhsT=pT16[:, bass.ts(qt, 128)],
                            rhs=va[:, h, c, :],
                            start=(c == 0 and qt == 0),
                            stop=(c == NCH - 1 and qt == 3),
                            skip_group_check=True,
                        )

                pend = []
                slot = 0
                for c in range(NCH):
                    for h in range(HPC):
                        hsl = slice(h * HD, (h + 1) * HD)
                        off = max(0, (c - PCH) * 128)
                        masked = c >= PCH
                        S = scp.tile([128, 512], FP32, tag="sc")
                        nc.tensor.matmul(
                            S[:, off:],
                            lhsT=kT[hsl, bass.ts(c, 128)],
                            rhs=qT[hsl, bsl][:, off:],
                            start=True, stop=not masked,
                        )
                        if masked:
                            nc.tensor.matmul(
                                S[:, off:],
                                lhsT=id16,
                                rhs=maskk[:, c - PCH, off:],
                                start=False, stop=True,
                                skip_group_check=True,
                            )
                        if len(pend) >= 11:
                            av(*pend.pop(0))
                        if slot % 16 in (2, 5, 8, 11, 14):  # DVE schraudolph exp
                            pTd = ptdp.tile([128, 512], I16, tag="pTd")
                            nc.vector.tensor_scalar(
                                pTd[:, off:], S[:, off:], SCH_A, SCH_B,
                                op0=mult, op1=add,
                            )
                            pT16 = pTd.bitcast(FP16)
                        else:  # ScalarE true exp
                            pT16 = ptap.tile([128, 512], FP16, tag="pTa")
                            nc.scalar.activation(
                                pT16[:, off:], S[:, off:], Exp, scale=SCALE
                            )
                        pend.append((c, h, pT16))
                        if filler and slot % 6 == 3:
                            filler.pop(0)()
                        slot += 1
                for pp in pend:
                    av(*pp)
                # divide (denominator in col 64 of each qt block), transpose to oT
                for h in range(HPC):
                    hsl = slice(h * HD, (h + 1) * HD)
                    for qt in range(4):
                        a = accs[h][:, qt * 65:qt * 65 + 65]
                        r = ottp.tile([128, 1], FP32, tag="r")
                        nc.vector.reciprocal(r, a[:, 64:65])
                        ot = ottp.tile([128, 64], FP16, tag="ott")
                        nc.vector.tensor_scalar(ot, a[:, :64], r, None, op0=mult)
                        tp = flexp.tile([64, 128], FP16, tag="flex")
                        nc.tensor.transpose(tp, ot, id16)
                        nc.vector.tensor_copy(
                            oT[hsl, b * T + qt * 128:b * T + (qt + 1) * 128], tp
                        )
                while filler:
                    filler.pop(0)()

            def o_proj_piece(b, tt):
                out_r = out[bass.ts(b, T), :].rearrange("(tt p) d -> p tt d", p=128)
                tsl = slice(b * T + tt * 128, b * T + (tt + 1) * 128)
                ost = ostp.tile([128, D], FP16, tag="ost")
                for nh in range(2):
                    ps = flexp.tile([128, 512], FP32, tag="flex")
                    nc.tensor.matmul(
                        ps, lhsT=oT[:, tsl], rhs=woT_s[:, bass.ts(nh, 512)],
                        start=True, stop=True,
                    )
                    nc.vector.tensor_copy(ost[:, bass.ts(nh, 512)], ps)
                nc.sync.dma_start(out_r[:, tt, :], ost)

            def o_proj_pieces(b):
                return [
                    (lambda tt: lambda: o_proj_piece(b, tt))(tt)
                    for tt in range(T // 128)
                ]

            for b in range(B):
                filler = []
                if b + 2 < B:
                    xT_s = dma_x(b + 2)
                    dma_cache(b + 2)
                    filler += proj_pieces(b + 2, xT_s)
                if b > 0:
                    filler += o_proj_pieces(b - 1)
                attn(b, filler)
            for piece in o_proj_pieces(B - 1):
                piece()

    nc.compile()
    return nc


def _prep(x, k_prev, v_prev, Wq, Wk, Wv, Wo):
    """Host-side shard + fp16 layout marshalling."""
    f = np.float32
    x2 = np.ascontiguousarray(np.asarray(x, f).reshape(TOK, D))
    xT = np.ascontiguousarray(x2.T).astype(F16NP)
    k_prev = np.asarray(k_prev, f)
    v_prev = np.asarray(v_prev, f)
    Wq, Wk, Wv, Wo = (np.asarray(w, f) for w in (Wq, Wk, Wv, Wo))

    def wpack(Wrows):  # [128, D] -> [128dp, ko, 128m]: w[dp,ko,m] = W[m, 128ko+dp]
        return np.ascontiguousarray(
            Wrows.T.reshape(D // 128, 128, 128).transpose(1, 0, 2)
        ).astype(F16NP)

    in_maps = []
    for c in range(NCORES):
        rows = slice(128 * c, 128 * (c + 1))
        hsl = slice(HPC * c, HPC * (c + 1))
        kT = np.ascontiguousarray(
            k_prev[:, hsl, :, :].transpose(0, 1, 3, 2)
        ).reshape(B, 128, PAST).astype(F16NP)
        vpk = np.empty((B, 128, HPC, PCH, HD + 1), F16NP)
        vpk[..., :HD] = v_prev[:, hsl, :, :].reshape(
            B, HPC, PCH, 128, HD
        ).transpose(0, 3, 1, 2, 4).astype(F16NP)
        vpk[..., HD] = 1.0
        in_maps.append(
            {
                "xT": xT,
                "wq": wpack(Wq[rows, :]),
                "wk": wpack(Wk[rows, :]),
                "wv": wpack(Wv[rows, :]),
                "woT": np.ascontiguousarray(Wo[:, rows].T).astype(F16NP),
                "kTp": kT,
                "vp": np.ascontiguousarray(vpk),
            }
        )
    return in_maps


def kernel(x, k_prev, v_prev, Wq, Wk, Wv, Wo):
    if "nc" not in _cache:
        _cache["nc"] = _build()
    nc = _cache["nc"]
    in_maps = _prep(x, k_prev, v_prev, Wq, Wk, Wv, Wo)
    res = run_bass_kernel_spmd(nc, in_maps, core_ids=list(range(NCORES)))
    acc = np.zeros((TOK, D), np.float64)
    for r in res.results:
        acc += r["out"]
    return acc.astype(np.float32).reshape(B, T, D)


# revision 34
# speedup vs baseline: 1.3127x; 1.0008x over previous
"""Multi-head attention with KV cache, sharded over 8 NeuronCores by head.

Problem (hardcoded shapes):
  x       [4, 512, 1024]      hidden states (B, T, D)
  k_prev  [4, 16, 3584, 64]   KV cache (B, H, PAST, HD)
  v_prev  [4, 16, 3584, 64]
  Wq/Wk/Wv/Wo [1024, 1024]    projection weights (torch Linear: y = x @ W.T)

Sharding: 16 heads / 8 cores = 2 heads per core (data stays full along batch).
Each core computes q/k/v projections for its 2 heads (column-parallel),
full attention for its heads, and a column-parallel o_proj partial
[2048, 1024] in fp16; the host sums the 8 partials (the o_proj all-reduce).

Device algorithm per core (fp16 matmul operands, fp32 PSUM accumulate,
measured end-to-end rel err ~8e-3 incl. the partial Schraudolph softmax):
  - q/k projections: W_slice @ x^T on PE, contracting D; evicted fp16 into
    qT [128, TOK] and per-batch k caches kT_b [128, L] (cache DMA'd fp16).
  - v projection computed PRE-TRANSPOSED (out[token, hd] per 128-token tile)
    directly into the va value cache [128keys, 2h, 32chunk, 65] whose 65th
    column is 1.0 (softmax denominator rides the AV matmul).
  - scores^T[key, q] = k @ q^T per 128-key chunk (K=HD=64), one chunk per
    1-bank PSUM tile on a ring of 4; both heads' chunk streams interleave so
    two softmax chains are always in flight.  Causal mask on the 4 newest
    chunks accumulates an fp16 identity @ mask matmul into the same group
    (dtype must match the scores matmul: a mid-group dtype or perf-mode
    switch faults the PE).
  - softmax: one exp instruction per chunk on ONE engine (PSUM dependency
    tracking is bank-granular; splitting a tile across engines serializes
    them).  ~11/16 of chunks use true exp on ScalarE (scale=1/sqrt(HD)
    folded in; scores are O(1), no max subtraction), 5/16 use a one-op
    bias-corrected Schraudolph exp on DVE (rint to int16, bitcast fp16,
    ~1.8% rms ripple that largely cancels in the softmax ratio).
  - AV TRANSPOSED: per (chunk, 128-query tile): acc[q, 0:65] += pT_tile^T @
    [v|1] -- all four query tiles accumulate in ONE psum bank as a single
    accumulation group (HW zeroes the 2KB zero-region on the first
    start=True and lazily zero-fills each byte's first write), N=65 per
    matmul so PE cost is half of the straight orientation.  AV matmuls are
    deferred 7 slots so PE's score stream never stalls on exp.  Divide is a
    per-partition reciprocal+multiply (denominator in column 64), then a
    PE transpose (fp16) back to oT [hd, tok].
  - o_proj column-parallel per batch; fp16 partial [2048, 1024] written out.
  Projections for batch b+2 and o_proj for batch b-1 are emitted as filler
  pieces inside batch b's attention loop to fill PE gaps.
"""

import numpy as np

import concourse.bass as bass
import concourse.mybir as mybir
import concourse.tile as tile
from concourse import bacc
from concourse.bass_utils import run_bass_kernel_spmd
from concourse.masks import make_identity

B, T, D = 4, 512, 1024
H, HD = 16, 64
PAST = 3584
L = PAST + T            # 4096 == MAX_CACHE, nothing is trimmed
SCALE = float(1.0 / np.sqrt(HD))
NCORES = 8
HPC = H // NCORES       # heads per core = 2
TOK = B * T             # 2048
NCH = L // 128          # 32 key chunks per (b, h)

PCH = PAST // 128       # 28 chunks from the cache
NPAIR = NCH // 2        # 16 chunk pairs (one exp instruction each)
FP32 = mybir.dt.float32
FP16 = mybir.dt.float16
I16 = mybir.dt.int16
NEG = -256.0            # mask added in psum units; exp(-256/8) == 0 in fp16
F16NP = np.float16

# Every third (chunk, head) slot computes softmax exp on DVE via a
# bias-corrected Schraudolph approximation (int16 rint -> bitcast fp16,
# ~1.8% rms multiplicative ripple); the rest use true exp on ScalarE.
# At key-fraction 1/3 this adds ~8e-3 end-to-end rel err (gate is 2e-2).
SCH_A = float((1024.0 / np.log(2.0)) * SCALE)
SCH_B = 15360.0 - 59.6

_cache = {}


def _build():
    nc = bacc.Bacc(None, target_bir_lowering=False)

    xT = nc.dram_tensor("xT", [D, TOK], FP16, kind="ExternalInput")
    wq = nc.dram_tensor("wq", [128, D // 128, 128], FP16, kind="ExternalInput")
    wk = nc.dram_tensor("wk", [128, D // 128, 128], FP16, kind="ExternalInput")
    wv = nc.dram_tensor("wv", [128, D // 128, 128], FP16, kind="ExternalInput")
    woT = nc.dram_tensor("woT", [128, D], FP16, kind="ExternalInput")
    kTp = nc.dram_tensor("kTp", [B, 128, PAST], FP16, kind="ExternalInput")
    vp = nc.dram_tensor("vp", [B, 128, HPC, PCH, HD + 1], FP16, kind="ExternalInput")
    out = nc.dram_tensor("out", [TOK, D], FP16, kind="ExternalOutput")

    Exp = mybir.ActivationFunctionType.Exp
    mult = mybir.AluOpType.mult
    add = mybir.AluOpType.add

    with tile.TileContext(nc) as tc:
        with (
            tc.tile_pool(name="const", bufs=1) as const,
            tc.tile_pool(name="persist", bufs=1) as persist,
            tc.tile_pool(name="xs", bufs=2) as xs,
            tc.tile_pool(name="pta", bufs=9) as ptap,
            tc.tile_pool(name="ptd", bufs=6) as ptdp,
            tc.tile_pool(name="ott", bufs=4) as ottp,
            tc.tile_pool(name="ost", bufs=3) as ostp,
            tc.tile_pool(name="sc_ps", bufs=4, space="PSUM") as scp,
            tc.tile_pool(name="acc_ps", bufs=2, space="PSUM") as accp,
            tc.tile_pool(name="flex_ps", bufs=2, space="PSUM") as flexp,
        ):
            # ---- constants ----
            identity = const.tile([128, 128], FP32)
            make_identity(nc, identity)
            id16 = const.tile([128, 128], FP16)
            nc.vector.tensor_copy(id16, identity)
            # fp16 causal masks, applied via an identity matmul accumulated
            # into the scores group (dtype must match the scores matmul: a
            # mid-group dtype/perf-mode switch faults the PE).
            # maskk[p, r, t] = NEG if t < 128r + p else 0
            maskk = const.tile([128, 4, T], FP16)
            nc.gpsimd.memset(maskk, 0.0)
            for r in range(4):
                nc.gpsimd.affine_select(
                    out=maskk[:, r, :], in_=maskk[:, r, :],
                    compare_op=mybir.AluOpType.is_ge,
                    fill=NEG, base=-(128 * r),
                    channel_multiplier=-1, pattern=[[1, T]],
                )
            ones_c = const.tile([128, 1], FP16)
            nc.gpsimd.memset(ones_c, 1.0)
            warm = const.tile([1, 1], FP32)
            nc.scalar.activation(warm, identity[:1, :1], Exp)

            # ---- persistent SBUF ----
            woT_s = persist.tile([128, D], FP16)
            qT = persist.tile([128, TOK], FP16, tag="qT")
            oT = persist.tile([128, TOK], FP16, tag="oT")
            w_s = {}
            for name, w in (("q", wq), ("k", wk), ("v", wv)):
                w_s[name] = persist.tile(
                    [128, D // 128, 128], FP16, tag=f"w{name}", name=f"w{name}"
                )
            kT_b = [
                persist.tile([128, L], FP16, tag=f"kT{b}", name=f"kT{b}")
                for b in range(B)
            ]
            va_b = [
                persist.tile([128, HPC, NCH, HD + 1], FP16, tag=f"va{b}",
                             name=f"va{b}")
                for b in range(B)
            ]

            xT_r = xT.rearrange("(ko p) t -> p ko t", p=128)

            def dma_cache(b):
                nc.sync.dma_start(kT_b[b][:, :PAST], kTp[b, :, :])
                nc.sync.dma_start(va_b[b][:, :, :PCH, :], vp[b, :, :, :, :])
                # ones column for the 4 new-v chunks
                nc.vector.tensor_copy(
                    va_b[b][:, :, PCH:, HD],
                    ones_c[:, :, None].to_broadcast([128, HPC, NCH - PCH]),
                )

            def dma_x(b, xT_s=None):
                if xT_s is None:
                    xT_s = xs.tile([128, D // 128, 512], FP16, tag="xT")
                half = D // 256
                nc.sync.dma_start(xT_s[:, :half, :], xT_r[:, :half, bass.ts(b, 512)])
                nc.sync.dma_start(xT_s[:, half:, :], xT_r[:, half:, bass.ts(b, 512)])
                return xT_s

            def proj_qk(b, xT_s, name):
                dst = qT[:, bass.ts(b, T)] if name == "q" else kT_b[b][:, PAST:]
                ps = flexp.tile([128, 512], FP32, tag="flex")
                for ko in range(D // 128):
                    nc.tensor.matmul(
                        ps, lhsT=w_s[name][:, ko, :], rhs=xT_s[:, ko, :],
                        start=(ko == 0), stop=(ko == D // 128 - 1),
                    )
                nc.vector.tensor_copy(dst, ps)

            def proj_v(b, xT_s, tt):
                ps = flexp.tile([128, 512], FP32, tag="flex")
                for ko in range(D // 128):
                    nc.tensor.matmul(
                        ps[:, :128],
                        lhsT=xT_s[:, ko, bass.ts(tt, 128)],
                        rhs=w_s["v"][:, ko, :],
                        start=(ko == 0), stop=(ko == D // 128 - 1),
                    )
                for h in range(HPC):
                    nc.vector.tensor_copy(
                        va_b[b][:, h, PCH + tt, :HD],
                        ps[:, h * HD:(h + 1) * HD],
                    )

            def proj_pieces(b, xT_s):
                return [
                    lambda: proj_qk(b, xT_s, "q"),
                    lambda: proj_qk(b, xT_s, "k"),
                ] + [
                    (lambda tt: lambda: proj_v(b, xT_s, tt))(tt)
                    for tt in range(T // 128)
                ]

            def proj(b, xT_s):
                for piece in proj_pieces(b, xT_s):
                    piece()

            # ---- phase A: caches + projections for b0/b1 ----
            nc.sync.dma_start(w_s["q"], wq[:, :, :])
            xT_s0 = dma_x(0)
            nc.sync.dma_start(w_s["k"], wk[:, :, :])
            nc.sync.dma_start(w_s["v"], wv[:, :, :])
            dma_cache(0)
            xT_s1 = dma_x(1)
            dma_cache(1)
            proj(0, xT_s0)
            proj(1, xT_s1)
            nc.sync.dma_start(woT_s, woT[:, :])

            # ---- phase B: attention, both heads' chunk streams interleaved ----
            # One score chunk per 1-bank psum tile (ring of 4); each chunk's
            # softmax exp is ONE instruction on ONE engine (PSUM dep tracking
            # is bank-granular, so any split of a tile across engines would
            # serialize them). Chunks go 2:1 to ScalarE (true exp) : DVE
            # (Schraudolph); AV matmuls are deferred several slots so they
            # never stall PE's score stream.
            def attn(b, filler):
                bsl = bass.ts(b, T)
                kT = kT_b[b]
                va = va_b[b]
                accs = [
                    accp.tile([128, 512], FP32, tag="acc", name=f"acc{b}_{h}")
                    for h in range(HPC)
                ]

                def av(c, h, pT16):
                    qt0 = max(0, c - PCH)  # first query tile this chunk sees
                    for qt in range(qt0, 4):
                        # One accumulation group for the whole bank: HW (like
                        # the sim) zeroes the full 2KB zero-region on the first
                        # start=True and lazily zero-fills each byte's first
                        # write, so all four qt sub-ranges share the group.
                        nc.tensor.matmul(
                            accs[h][:, qt * 65:qt * 65 + 65],
                            lhsT=pT16[:, bass.ts(qt, 128)],
                            rhs=va[:, h, c, :],
                            start=(c == 0 and qt == 0),
                            stop=(c == NCH - 1 and qt == 3),
                            skip_group_check=True,
                        )

                pend = []
                slot = 0
                for c in range(NCH):
                    for h in range(HPC):
                        hsl = slice(h * HD, (h + 1) * HD)
                        off = max(0, (c - PCH) * 128)
                        masked = c >= PCH
                        S = scp.tile([128, 512], FP32, tag="sc")
                        nc.tensor.matmul(
                            S[:, off:],
                            lhsT=kT[hsl, bass.ts(c, 128)],
                            rhs=qT[hsl, bsl][:, off:],
                            start=True, stop=not masked,
                        )
                        if masked:
                            nc.tensor.matmul(
                                S[:, off:],
                                lhsT=id16,
                                rhs=maskk[:, c - PCH, off:],
                                start=False, stop=True,
                                skip_group_check=True,
                            )
                        if len(pend) >= 13:
                            av(*pend.pop(0))
                        if slot % 16 in (2, 5, 8, 11, 14):  # DVE schraudolph exp
                            pTd = ptdp.tile([128, 512], I16, tag="pTd")
                            nc.vector.tensor_scalar(
                                pTd[:, off:], S[:, off:], SCH_A, SCH_B,
                                op0=mult, op1=add,
                            )
                            pT16 = pTd.bitcast(FP16)
                        else:  # ScalarE true exp
                            pT16 = ptap.tile([128, 512], FP16, tag="pTa")
                            nc.scalar.activation(
                                pT16[:, off:], S[:, off:], Exp, scale=SCALE
                            )
                        pend.append((c, h, pT16))
                        if filler and slot % 6 == 3:
                            filler.pop(0)()
                        slot += 1
                # divide (denominator in col 64 of each qt block), transpose
                # to oT; emitted per head as soon as its last AV drains so the
                # next batch's accumulator slots free earlier
                def divide_head(h):
                    hsl = slice(h * HD, (h + 1) * HD)
                    for qt in range(4):
                        a = accs[h][:, qt * 65:qt * 65 + 65]
                        r = ottp.tile([128, 1], FP32, tag="r")
                        nc.vector.reciprocal(r, a[:, 64:65])
                        ot = ottp.tile([128, 64], FP16, tag="ott")
                        nc.vector.tensor_scalar(ot, a[:, :64], r, None, op0=mult)
                        tp = flexp.tile([64, 128], FP16, tag="flex")
                        nc.tensor.transpose(tp, ot, id16)
                        nc.vector.tensor_copy(
                            oT[hsl, b * T + qt * 128:b * T + (qt + 1) * 128], tp
                        )

                last = {h: max(i for i, pp in enumerate(pend) if pp[1] == h)
                        for h in range(HPC)}
                for i, pp in enumerate(pend):
                    av(*pp)
                    for h in range(HPC):
                        if last[h] == i:
                            divide_head(h)
                while filler:
                    filler.pop(0)()

            def o_proj_piece(b, tt):
                out_r = out[bass.ts(b, T), :].rearrange("(tt p) d -> p tt d", p=128)
                tsl = slice(b * T + tt * 128, b * T + (tt + 1) * 128)
                ost = ostp.tile([128, D], FP16, tag="ost")
                for nh in range(2):
                    ps = flexp.tile([128, 512], FP32, tag="flex")
                    nc.tensor.matmul(
                        ps, lhsT=oT[:, tsl], rhs=woT_s[:, bass.ts(nh, 512)],
                        start=True, stop=True,
                    )
                    nc.vector.tensor_copy(ost[:, bass.ts(nh, 512)], ps)
                nc.sync.dma_start(out_r[:, tt, :], ost)

            def o_proj_pieces(b):
                return [
                    (lambda tt: lambda: o_proj_piece(b, tt))(tt)
                    for tt in range(T // 128)
                ]

            for b in range(B):
                filler = []
                if b + 2 < B:
                    xT_s = dma_x(b + 2)
                    dma_cache(b + 2)
                    filler += proj_pieces(b + 2, xT_s)
                if b > 0:
                    filler += o_proj_pieces(b - 1)
                attn(b, filler)
            for piece in o_proj_pieces(B - 1):
                piece()

    nc.compile()
    return nc


def _prep(x, k_prev, v_prev, Wq, Wk, Wv, Wo):
    """Host-side shard + fp16 layout marshalling."""
    f = np.float32
    x2 = np.ascontiguousarray(np.asarray(x, f).reshape(TOK, D))
    xT = np.ascontiguousarray(x2.T).astype(F16NP)
    k_prev = np.asarray(k_prev, f)
    v_prev = np.asarray(v_prev, f)
    Wq, Wk, Wv, Wo = (np.asarray(w, f) for w in (Wq, Wk, Wv, Wo))

    def wpack(Wrows):  # [128, D] -> [128dp, ko, 128m]: w[dp,ko,m] = W[m, 128ko+dp]
        return np.ascontiguousarray(
            Wrows.T.reshape(D // 128, 128, 128).transpose(1, 0, 2)
        ).astype(F16NP)

    in_maps = []
    for c in range(NCORES):
        rows = slice(128 * c, 128 * (c + 1))
        hsl = slice(HPC * c, HPC * (c + 1))
        kT = np.ascontiguousarray(
            k_prev[:, hsl, :, :].transpose(0, 1, 3, 2)
        ).reshape(B, 128, PAST).astype(F16NP)
        vpk = np.empty((B, 128, HPC, PCH, HD + 1), F16NP)
        vpk[..., :HD] = v_prev[:, hsl, :, :].reshape(
            B, HPC, PCH, 128, HD
        ).transpose(0, 3, 1, 2, 4).astype(F16NP)
        vpk[..., HD] = 1.0
        in_maps.append(
            {
                "xT": xT,
                "wq": wpack(Wq[rows, :]),
                "wk": wpack(Wk[rows, :]),
                "wv": wpack(Wv[rows, :]),
                "woT": np.ascontiguousarray(Wo[:, rows].T).astype(F16NP),
                "kTp": kT,
                "vp": np.ascontiguousarray(vpk),
            }
        )
    return in_maps


def kernel(x, k_prev, v_prev, Wq, Wk, Wv, Wo):
    if "nc" not in _cache:
        _cache["nc"] = _build()
    nc = _cache["nc"]
    in_maps = _prep(x, k_prev, v_prev, Wq, Wk, Wv, Wo)
    res = run_bass_kernel_spmd(nc, in_maps, core_ids=list(range(NCORES)))
    acc = np.zeros((TOK, D), np.float64)
    for r in res.results:
        acc += r["out"]
    return acc.astype(np.float32).reshape(B, T, D)
